# revision 33
# baseline (speedup 1.0000x reference)
"""Trainium2 Bass kernel for nn_Attention_86423331930617.

Reference math (per batch b of 16):
  frate = [framerate[b], resolution[b]]                       # [2]
  h  = ((frate@W1+b1)@W2+b2)@W3+b3                            # [98304]
  qkvw = softmax(h.reshape(128, 768), axis=0)                 # over dim d
  wq, wk, wv = split(qkvw, 3, -1)                             # [128, 256] each
  q/k/v = x[b] @ w*      -> heads [8, 600, 32]
  dots = q@k.T / sqrt(32); attn = softmax(dots, -1) * mask[b]
  out  = attn @ v -> [600, 256]
  ow   = softmax(((frate@V1+c1)@V2+c2)@V3+c3 .reshape(256,128), axis=0)
  y    = out @ ow                                             # [600, 128]

Distribution over 8 cores (single NEFF, three AllToAlls):
  - warmup 256B AllToAll at t~0 absorbs the collective rendezvous
    barrier while the hypernet computes.
  - Hypernet big matmuls column-sharded (fp8 DoubleRow ks-pairs);
    A2A #1 ships q/k right after the 16 qk chunks; A2A #2 ships wv + ow
    and overlaps attention part 1.
  - Attention batch-sharded: core c does batches 2c, 2c+1.
DMA discipline: hypernet stores issue on the gpsimd queue (in front of
the collectives they feed), x/mask loads on the vector queue, weights +
a2a unpacks on sync. Unpacks/mask are single strided DMAs.
Packing tricks (tile_position inferred from out-AP base partition):
  - hypernet h-chunks: 4x [16,512] packed per PSUM bank -> 1 ACT copy
  - rowsums: ones[m,32] stationary -> [32,n] broadcast, 4 heads/bank
  - attn out (po): 4 heads/bank -> single TT applies 1/den and lands
    directly in the y-matmul operand layout
Softmax normalizers fold into ACT scales; exp needs no max-subtraction
(hypernet outputs and dots are O(0.1) by construction).
"""
import sys

sys.path.insert(0, "/opt/trn_rl_repo")
if "/root/.axon_site" not in sys.path:
    sys.path.insert(0, "/root/.axon_site")

import numpy as np
import ml_dtypes

import concourse.bass as bass
import concourse.mybir as mybir
import concourse.tile as tile
from concourse.vector_clock import ScopedClock
from concourse.bass_utils import run_bass_kernel_spmd

F32 = mybir.dt.float32
BF16 = mybir.dt.bfloat16
BF16_NP = ml_dtypes.bfloat16
FP8 = mybir.dt.float8e4
FP8_NP = mybir.dt.np(mybir.dt.float8e4)
DR = mybir.MatmulPerfMode.DoubleRow
W3_SCALE = 64.0
A2_SCALE = 4.0
UNSCALE = 1.0 / (W3_SCALE * A2_SCALE)
EXP = mybir.ActivationFunctionType.Exp
IDENT = mybir.ActivationFunctionType.Identity
MULT = mybir.AluOpType.mult

NCORES = 8
B, N, DIM, HEADS, DH = 16, 600, 128, 8, 32
INNER = HEADS * DH          # 256
D3 = 3 * DIM                # 384
E3 = 3 * INNER              # 768
BPC = B // NCORES           # batches per core = 2
W3_SL = 16 * E3             # 12288 w3 cols per core (16 d-rows)
V3_SL = 32 * DIM            # 4096 v3 cols per core (32 i-rows)
SCALE = DH ** -0.5
NH = [(0, 300), (300, 300)]                       # n halves
MT = [(0, 128), (128, 128), (256, 128), (384, 128), (512, 88)]  # m tiles
CH = 512                    # hypernet chunk cols
QK_W = 16 * 512             # 8192: q/k cols per core slice (a2a #1)
VB_W = 16 * 256             # 4096: wv cols per core slice (a2a #2)
A2_W = VB_W + V3_SL         # 8192: a2a #2 width (wv + ow)


# ---------------------------------------------------------------------------
# This walrus build accepts at most ONE sync wait / update per instruction;
# Tile can emit more. Split extras onto adjacent same-engine NoOps.
class _SplitWaitTileContext(tile.TileContext):
    def _split_sync(self, insts):
        out = []
        for inst in insts:
            si = inst.sync_info
            if si is None:
                out.append(inst)
                continue
            waits = list(si.on_wait) if si.on_wait else []
            updates = list(si.on_update) if si.on_update else []
            if len(waits) <= 1 and len(updates) <= 1:
                out.append(inst)
                continue
            for w in waits[1:]:
                nop = mybir.InstNoOp(name=f"I-{self.nc.next_id()}", ins=[], outs=[])
                nop.engine = inst.engine
                nop.sync_info = mybir.SyncInfo(on_wait=[w], on_update=[])
                out.append(nop)
            inst.sync_info = mybir.SyncInfo(on_wait=waits[:1], on_update=updates[:1])
            out.append(inst)
            for u in updates[1:]:
                nop = mybir.InstNoOp(name=f"I-{self.nc.next_id()}", ins=[], outs=[])
                nop.engine = inst.engine
                nop.sync_info = mybir.SyncInfo(on_wait=[], on_update=[u])
                out.append(nop)
        return out

    def _lower_ordered_insts(self, ordered):
        for bb_name in list(ordered.keys()):
            ordered[bb_name] = self._split_sync(ordered[bb_name])
        return super()._lower_ordered_insts(ordered)

    def _drain_and_barrier(self, tick_clock, wait_clock):
        nc = self.nc
        probe = nc.sync.nop()
        wait_clock.add_sem_waits(probe.ins, ScopedClock({None: tick_clock.global_clock}))
        si = probe.ins.sync_info
        waits = list(si.on_wait) if si is not None and si.on_wait else []
        if len(waits) > 1:
            probe.ins.sync_info = mybir.SyncInfo(on_wait=waits[:1], on_update=[])
            for w in waits[1:]:
                extra = nc.sync.nop()
                extra.ins.sync_info = mybir.SyncInfo(on_wait=[w], on_update=[])
        nc.sync.drain()
        nc.all_engine_barrier()
        assert self.sems is not None
        popped = nc._tile_sem_poison_stack.pop()
        assert popped is self._sem_poison
        nc.clear_and_free_semaphores(list(self.sems.allocated().values()))
        nc.all_engine_barrier()


# ---------------------------------------------------------------------------
def _build_program(with_bias):
    """Emit the per-core SPMD program. with_bias: (b12, c12, b3, c3) flags."""
    b12, c12, b3f, c3f = with_bias
    nc = bass.Bass("TRN2", target_bir_lowering=False, debug=False,
                   num_devices=NCORES)

    xT = nc.dram_tensor("xT", [BPC, DIM, N], BF16, kind="ExternalInput")
    maskT = nc.dram_tensor("maskT", [BPC, 5, DIM, N], BF16, kind="ExternalInput")
    frateT = nc.dram_tensor("frateT", [2, B], F32, kind="ExternalInput")
    w1 = nc.dram_tensor("w1", [2, D3], F32, kind="ExternalInput")
    w2 = nc.dram_tensor("w2", [D3, D3], F32, kind="ExternalInput")
    w3c = nc.dram_tensor("w3c", [D3, W3_SL], FP8, kind="ExternalInput")
    v1 = nc.dram_tensor("v1", [2, INNER], F32, kind="ExternalInput")
    v2 = nc.dram_tensor("v2", [INNER, INNER], F32, kind="ExternalInput")
    v3c = nc.dram_tensor("v3c", [INNER, V3_SL], FP8, kind="ExternalInput")
    if b12:
        b1t = nc.dram_tensor("b1t", [D3, 1], F32, kind="ExternalInput")
        b2t = nc.dram_tensor("b2t", [D3, 1], F32, kind="ExternalInput")
    if c12:
        c1t = nc.dram_tensor("c1t", [INNER, 1], F32, kind="ExternalInput")
        c2t = nc.dram_tensor("c2t", [INNER, 1], F32, kind="ExternalInput")
    if b3f:
        b3c = nc.dram_tensor("b3c", [1, W3_SL], BF16, kind="ExternalInput")
    if c3f:
        c3c = nc.dram_tensor("c3c", [1, V3_SL], BF16, kind="ExternalInput")
    yT = nc.dram_tensor("yT", [BPC, DIM, N], BF16, kind="ExternalOutput")

    with _SplitWaitTileContext(nc) as tc:
        with (
            tc.tile_pool(name="const", bufs=1) as cpool,
            tc.tile_pool(name="wts", bufs=1) as wpool,
            tc.tile_pool(name="achain", bufs=1) as apool,
            tc.tile_pool(name="hcopy", bufs=6) as hpool,
            tc.tile_pool(name="batch", bufs=1) as bpool,
            tc.tile_pool(name="etile", bufs=30) as epool,
            tc.tile_pool(name="rden", bufs=8) as rpool,
            # PSUM budget (8 banks of 2KB/partition):
            #   psA 3x2 banks (dots/a-chain/proj/y), psH 1 (hypernet chunk
            #   strips; ctx-freed after phase A for psD), psB 1 (rowsum
            #   packs, po packs, colsums; ping-pongs with psD in attention)
            tc.tile_pool(name="psA", bufs=3, space="PSUM") as psA,
            tc.tile_pool(name="psB", bufs=1, space="PSUM") as psB,
            tc.tile_pool(name="dram", bufs=1, space="DRAM") as dpool,
        ):
            # ---- warmup collective: absorbs the cross-core rendezvous
            # barrier while the hypernet phase computes. The whole chain
            # lives on the gpsimd queue so it issues at t~0.
            warm_sb = cpool.tile([8, 16], BF16, name="warm_sb")
            nc.gpsimd.memset(warm_sb[:], 0.0)
            warm_in = dpool.tile([8, 16], BF16, name="warm_in")
            nc.gpsimd.dma_start(out=warm_in[:], in_=warm_sb[:])
            warm_out = dpool.tile([8, 16], BF16, name="warm_out")
            nc.gpsimd.collective_compute(
                "AllToAll", mybir.AluOpType.bypass,
                replica_groups=[list(range(NCORES))],
                ins=[warm_in[:]], outs=[warm_out[:]],
            )

            # ---- constants
            ones_col = cpool.tile([DIM, 1], BF16, name="ones_col")
            nc.vector.memset(ones_col[:], 1.0)
            ones32 = cpool.tile([DIM, 32], BF16, name="ones32")
            nc.vector.memset(ones32[:], 1.0)
            if b3f or c3f:
                ones_row16 = cpool.tile([1, B], BF16, name="ones_row16")
                nc.vector.memset(ones_row16[:], 1.0)

            # ---- small weights in (sync queue)
            fr_sb = wpool.tile([2, B], F32, name="fr_sb")
            nc.sync.dma_start(out=fr_sb[:], in_=frateT[:])
            w1_sb = wpool.tile([2, D3], F32, name="w1_sb")
            nc.sync.dma_start(out=w1_sb[:], in_=w1[:])
            w2_sb = wpool.tile([DIM, 3, D3], F32, name="w2_sb")
            nc.sync.dma_start(out=w2_sb[:],
                              in_=w2[:].rearrange("(k p) n -> p k n", p=DIM))
            v1_sb = wpool.tile([2, INNER], F32, name="v1_sb")
            nc.sync.dma_start(out=v1_sb[:], in_=v1[:])
            v2_sb = wpool.tile([DIM, 2, INNER], F32, name="v2_sb")
            nc.sync.dma_start(out=v2_sb[:],
                              in_=v2[:].rearrange("(k p) n -> p k n", p=DIM))
            bias_sb = {}
            if b12:
                bias_sb["b1"] = wpool.tile([D3, 1], F32, name="b1_sb")
                nc.sync.dma_start(out=bias_sb["b1"][:], in_=b1t[:])
                bias_sb["b2"] = wpool.tile([D3, 1], F32, name="b2_sb")
                nc.sync.dma_start(out=bias_sb["b2"][:], in_=b2t[:])
            if c12:
                bias_sb["c1"] = wpool.tile([INNER, 1], F32, name="c1_sb")
                nc.sync.dma_start(out=bias_sb["c1"][:], in_=c1t[:])
                bias_sb["c2"] = wpool.tile([INNER, 1], F32, name="c2_sb")
                nc.sync.dma_start(out=bias_sb["c2"][:], in_=c2t[:])
            if b3f:
                b3_sb = wpool.tile([1, W3_SL], BF16, name="b3_sb")
                nc.sync.dma_start(out=b3_sb[:], in_=b3c[:])
            if c3f:
                c3_sb = wpool.tile([1, V3_SL], BF16, name="c3_sb")
                nc.sync.dma_start(out=c3_sb[:], in_=c3c[:])

            # ---- big hypernet weight slices (freed before phase B needs
            # the masked-exp tile pool)
            w3ctx = tc.tile_pool(name="w3", bufs=1)
            w3pool = w3ctx.__enter__()
            w3dr = w3pool.tile([DIM, 3, W3_SL], FP8, name="w3dr")
            w3view = w3c[:].rearrange("(ks p) n -> p ks n", p=DIM)
            for q4 in range(4):
                q0 = q4 * (W3_SL // 4)
                nc.sync.dma_start(out=w3dr[:, :, q0:q0 + W3_SL // 4],
                                  in_=w3view[:, :, q0:q0 + W3_SL // 4])
            v3dr = w3pool.tile([DIM, 2, V3_SL], FP8, name="v3dr")
            nc.sync.dma_start(
                out=v3dr[:], in_=v3c[:].rearrange("(ks p) n -> p ks n", p=DIM))

            # ---- a-chain: a1T = (frate@W1+b1).T as 3x[128,16]
            a1T = []
            for t in range(3):
                p = psA.tile([DIM, 1024], F32, name="pa", tag="pA")
                nc.tensor.matmul(p[:, :B], w1_sb[:, 128 * t:128 * (t + 1)],
                                 fr_sb[:], start=True, stop=True)
                s = apool.tile([DIM, B], F32, name=f"a1T{t}")
                if b12:
                    nc.scalar.activation(s[:], p[:, :B], IDENT,
                                         bias=bias_sb["b1"][128 * t:128 * (t + 1), :])
                else:
                    nc.scalar.copy(s[:], p[:, :B])
                a1T.append(s)
            a2f8 = apool.tile([DIM, 3, B], FP8, name="a2f8")
            for t in range(3):
                p = psA.tile([DIM, 1024], F32, name="pa2", tag="pA")
                for k in range(3):
                    nc.tensor.matmul(p[:, :B], w2_sb[:, k, 128 * t:128 * (t + 1)],
                                     a1T[k][:], start=(k == 0), stop=(k == 2))
                if b12:
                    tmp = apool.tile([DIM, B], F32, name=f"a2tmp{t}")
                    nc.scalar.activation(tmp[:], p[:, :B], IDENT,
                                         bias=bias_sb["b2"][128 * t:128 * (t + 1), :])
                    with nc.allow_low_precision("fp8 hypernet activations"):
                        nc.vector.tensor_scalar_mul(a2f8[:, t, :], tmp[:],
                                                    A2_SCALE)
                else:
                    nc.scalar.mul(a2f8[:, t, :], p[:, :B], A2_SCALE)
            av1T = []
            for t in range(2):
                p = psA.tile([DIM, 1024], F32, name="pav", tag="pA")
                nc.tensor.matmul(p[:, :B], v1_sb[:, 128 * t:128 * (t + 1)],
                                 fr_sb[:], start=True, stop=True)
                s = apool.tile([DIM, B], F32, name=f"av1T{t}")
                if c12:
                    nc.scalar.activation(s[:], p[:, :B], IDENT,
                                         bias=bias_sb["c1"][128 * t:128 * (t + 1), :])
                else:
                    nc.scalar.copy(s[:], p[:, :B])
                av1T.append(s)
            avf8 = apool.tile([DIM, 2, B], FP8, name="avf8")
            for t in range(2):
                p = psA.tile([DIM, 1024], F32, name="pav2", tag="pA")
                for k in range(2):
                    nc.tensor.matmul(p[:, :B], v2_sb[:, k, 128 * t:128 * (t + 1)],
                                     av1T[k][:], start=(k == 0), stop=(k == 1))
                if c12:
                    tmp = apool.tile([DIM, B], F32, name=f"avtmp{t}")
                    nc.scalar.activation(tmp[:], p[:, :B], IDENT,
                                         bias=bias_sb["c2"][128 * t:128 * (t + 1), :])
                    with nc.allow_low_precision("fp8 hypernet activations"):
                        nc.vector.tensor_scalar_mul(avf8[:, t, :], tmp[:],
                                                    A2_SCALE)
                else:
                    nc.scalar.mul(avf8[:, t, :], p[:, :B], A2_SCALE)

            # ---- x / mask inputs on the scalar queue, emitted after the
            # a-chain so its ACT ops aren't queued behind DMA dispatches
            xT_sb = [bpool.tile([DIM, N], BF16, name=f"xT_sb{i}")
                     for i in range(BPC)]
            for i in range(BPC):
                nc.scalar.dma_start(out=xT_sb[i][:], in_=xT[i])
            maskT_sb = [bpool.tile([DIM, 5, N], BF16, name=f"mask_sb{i}")
                        for i in range(BPC)]
            for i in range(BPC):
                nc.scalar.dma_start(
                    out=maskT_sb[i][:],
                    in_=maskT[i].rearrange("mt p n -> p mt n"))

            # ---- big hypernet matmuls -> two a2a inputs
            # host reorders w3c cols: first 16x512 (d-major, e<512 = q,k),
            # then 16x256 (e>=512 = wv). a2a #1 ships q/k right after the
            # qk chunks; a2a #2 ships wv + ow and overlaps attention part 1.
            # Chunks packed 4-per-bank at partition strips {0,32,64,96};
            # one ACT copy per 4 chunks. Output stays scaled by 1/UNSCALE;
            # consumers fold UNSCALE into their exp() scale. Stores issue on
            # the gpsimd queue in front of the collectives they feed.
            a2a1_in = dpool.tile([B, QK_W], BF16, name="a2a1_in")
            a2a2_in = dpool.tile([B, A2_W], BF16, name="a2a2_in")
            psHctx = tc.tile_pool(name="psH", bufs=1, space="PSUM")
            psH = psHctx.__enter__()

            def w3_group(g, tgt, toff, n_chunks=4):
                """Chunks 4g..4g+n_chunks of the 24 w3 chunks (512 cols)."""
                p = psH.tile([128, CH], F32, name="ph", tag="pH")
                for c in range(n_chunks):
                    j = 4 * g + c
                    st = 32 * c
                    for ks in range(3):
                        nc.tensor.matmul(p[st:st + B, :], a2f8[:, ks, :],
                                         w3dr[:, ks, CH * j:CH * (j + 1)],
                                         start=(ks == 0),
                                         stop=(ks == 2 and not b3f),
                                         tile_position=(0, st))
                    if b3f:
                        nc.tensor.matmul(p[st:st + B, :], ones_row16[:],
                                         b3_sb[:, CH * j:CH * (j + 1)],
                                         start=False, stop=True,
                                         tile_position=(0, st))
                s = hpool.tile([128, CH], BF16, name="hs", tag="hs")
                nc.scalar.copy(s[:], p[:])
                for c in range(n_chunks):
                    j = 4 * g + c
                    nc.gpsimd.dma_start(
                        out=tgt[:, CH * j - toff:CH * (j + 1) - toff],
                        in_=s[32 * c:32 * c + B, :])

            for g in range(4):          # qk chunks 0..15
                w3_group(g, a2a1_in, 0)
            a2a1_out = dpool.tile([B, QK_W], BF16, name="a2a1_out")
            nc.gpsimd.collective_compute(
                "AllToAll", mybir.AluOpType.bypass,
                replica_groups=[list(range(NCORES))],
                ins=[a2a1_in[:]], outs=[a2a1_out[:]],
            )
            for g in range(4, 6):       # wv chunks 16..23 -> a2a2 cols 0..4096
                w3_group(g, a2a2_in, QK_W)
            for g in range(2):          # v3: 8 chunks of 512 -> a2a #2 (ow)
                p = psH.tile([128, CH], F32, name="phv", tag="pH")
                for c in range(4):
                    j = 4 * g + c
                    st = 32 * c
                    for ks in range(2):
                        nc.tensor.matmul(p[st:st + B, :], avf8[:, ks, :],
                                         v3dr[:, ks, CH * j:CH * (j + 1)],
                                         start=(ks == 0),
                                         stop=(ks == 1 and not c3f),
                                         tile_position=(0, st))
                    if c3f:
                        nc.tensor.matmul(p[st:st + B, :], ones_row16[:],
                                         c3_sb[:, CH * j:CH * (j + 1)],
                                         start=False, stop=True,
                                         tile_position=(0, st))
                s = hpool.tile([128, CH], BF16, name="hvs", tag="hs")
                nc.scalar.copy(s[:], p[:])
                for c in range(4):
                    j = 4 * g + c
                    nc.gpsimd.dma_start(
                        out=a2a2_in[:, VB_W + CH * j:VB_W + CH * (j + 1)],
                        in_=s[32 * c:32 * c + B, :])
            psHctx.__exit__(None, None, None)
            # psH's bank is free once phase A drains; attention ping-pongs
            # rowsum/po packs between psB and psD so a pack's matmuls never
            # wait on the previous pack's DVE drain.
            psDctx = tc.tile_pool(name="psD", bufs=1, space="PSUM")
            psD = psDctx.__enter__()
            a2a2_out = dpool.tile([B, A2_W], BF16, name="a2a2_out")
            nc.gpsimd.collective_compute(
                "AllToAll", mybir.AluOpType.bypass,
                replica_groups=[list(range(NCORES))],
                ins=[a2a2_in[:]], outs=[a2a2_out[:]],
            )
            w3ctx.__exit__(None, None, None)
            emctx = tc.tile_pool(name="emk", bufs=62)
            em_pool = emctx.__enter__()
            # row (2s+i) holds my batch i's hypernet outputs from source s
            h1view = a2a1_out[:].rearrange(
                "(s two) (d e) -> two s d e", two=BPC, d=16)
            h2view = a2a2_out[:, :VB_W].rearrange(
                "(s two) (d e) -> two s d e", two=BPC, d=16)
            hvview = a2a2_out[:, VB_W:].rearrange(
                "(s two) (iv dd) -> two s iv dd", two=BPC, iv=32)

            # ================= attention =================
            # part1(b0) -> part2-pre(b0) -> part1(b1) with b0's attn@v
            # chains injected between heads (fills PE gaps while ACT paces
            # the exps) -> y(b0) -> part2(b1).
            st_ = [dict() for _ in range(BPC)]

            def part1(i, inject=None):
                s_i = st_[i]
                # lead-in pipelined by e-half: the q half (packs 0,1) flows
                # dma->exp->colsum->recip->proj before the k half's exp.
                qrawA = bpool.tile([DIM, 512], BF16, name="qrawA", tag="qrawA")
                for eh in range(2):
                    nc.gpsimd.dma_start(
                        out=qrawA[:, 256 * eh:256 * (eh + 1)],
                        in_=h1view[i][:, :, 256 * eh:256 * (eh + 1)])
                ehqA = bpool.tile([DIM, 512], BF16, name=f"ehqA{i}",
                                  tag=f"ehqA{i}")
                pcs = psB.tile([DIM, 512], F32, name="pcs", tag="pB")
                # recipA col p = softmax denom recip for e in [128p,128p+128)
                # == per-partition scale for qkT pack p (strips match).
                recipA = bpool.tile([DIM, 4], F32, name="recipA", tag="recipA")
                qkT = [None] * 4

                def lead_half(eh):
                    nc.scalar.activation(ehqA[:, 256 * eh:256 * (eh + 1)],
                                         qrawA[:, 256 * eh:256 * (eh + 1)],
                                         EXP, scale=UNSCALE)
                    for j in (2 * eh, 2 * eh + 1):
                        nc.tensor.matmul(pcs[:, j:j + 1],
                                         ehqA[:, 128 * j:128 * (j + 1)],
                                         ones_col[:], start=True, stop=True)
                    nc.vector.reciprocal(recipA[:, 2 * eh:2 * eh + 2],
                                         pcs[:, 2 * eh:2 * eh + 2])
                    if eh == 0:
                        nc.vector.tensor_scalar_mul(recipA[:, 0:2],
                                                    recipA[:, 0:2], SCALE)

                def proj_pack(pk):
                    pp = psA.tile([DIM, 1024], F32, name="pproj", tag="pA")
                    ppv = pp[:].rearrange("p (b n) -> p b n", b=2)
                    for st in range(2):
                        blk = 2 * pk + st
                        for hf, (n0, nsz) in enumerate(NH):
                            nc.tensor.matmul(
                                ppv[64 * st:64 * st + 64, hf, :nsz],
                                ehqA[:, 64 * blk:64 * blk + 64],
                                xT_sb[i][:, n0:n0 + nsz],
                                start=True, stop=True,
                                tile_position=(0, 64 * st))
                    s = bpool.tile([DIM, N], BF16, name=f"qkT{pk}",
                                   tag=f"qkT{i}_{pk}")
                    sview = s[:].rearrange("p (hf n) -> p hf n", hf=2)
                    nc.scalar.activation(
                        sview,
                        pp[:].rearrange("p (hf n) -> p hf n", hf=2)[:, :, :300],
                        mybir.ActivationFunctionType.Copy,
                        scale=recipA[:, pk:pk + 1])
                    qkT[pk] = s

                # heads 0-3 consume packs (0, 2): emit those first
                lead_half(0)
                proj_pack(0)
                lead_half(1)
                proj_pack(2)
                proj_pack(1)
                proj_pack(3)
                em_all, em_eun, rden_all = {}, {}, {}
                s_i["em"], s_i["rden"] = em_all, rden_all

                def emit_rowsum_burst(kg):
                    # 20 contiguous full-K PE matmuls per half
                    for hf, (n0, nsz) in enumerate(NH):
                        pool_, tag_ = ((psB, "pB") if (2 * kg + hf) % 2 == 0
                                       else (psD, "pD"))
                        prs = pool_.tile([128, 512], F32, name="prs", tag=tag_)
                        for j in range(4):
                            h = 4 * kg + j
                            for mt, (m0, msz) in enumerate(MT):
                                nc.tensor.matmul(
                                    prs[32 * j:32 * j + 32, :nsz],
                                    ones32[:msz, :],
                                    em_eun[(h, mt)][:msz, n0:n0 + nsz],
                                    start=(mt == 0), stop=(mt == 4),
                                    tile_position=(0, 32 * j))
                        rden = rpool.tile([128, 300], F32, name="rden",
                                          tag="rden")
                        nc.vector.reciprocal(rden[:], prs[:, :300])
                        rden_all[(kg, hf)] = rden

                for h in range(HEADS):
                    pk, ro = h // 4, 32 * (h % 4)
                    qp, kp = qkT[pk], qkT[2 + pk]
                    for mt, (m0, msz) in enumerate(MT):
                        pd = psA.tile([128, 1024], F32, name="pdots", tag="pA")
                        pdv = pd[:].rearrange("p (b n) -> p b n", b=2)
                        for hf, (n0, nsz) in enumerate(NH):
                            nc.tensor.matmul(
                                pdv[:msz, hf, :nsz],
                                kp[ro:ro + 32, m0:m0 + msz],
                                qp[ro:ro + 32, n0:n0 + nsz],
                                start=True, stop=True,
                                tile_position=(ro, 0))
                        e_t = epool.tile([128, N], BF16, name="e_t", tag="e")
                        nc.scalar.activation(
                            e_t[:msz].rearrange("p (hf n) -> p hf n", hf=2),
                            pd[:msz].rearrange(
                                "p (hf n) -> p hf n", hf=2)[:, :, :300],
                            EXP)
                        em_t = em_pool.tile([128, N], BF16, name="em_t",
                                            tag="em")
                        nc.vector.tensor_mul(em_t[:msz], e_t[:msz],
                                             maskT_sb[i][:msz, mt, :])
                        em_eun[(h, mt)] = e_t
                        em_all[(h, mt)] = em_t
                    if h == 4:
                        emit_rowsum_burst(0)
                    if inject is not None:
                        inject(h)
                emit_rowsum_burst(1)

            def part2_pre(i):
                s_i = st_[i]
                qrawB = bpool.tile([DIM, 256], BF16, name="qrawB", tag="qrawB")
                nc.gpsimd.dma_start(out=qrawB[:], in_=h2view[i])
                ehqB = bpool.tile([DIM, 256], BF16, name=f"ehqB{i}",
                                  tag=f"ehqB{i}")
                nc.scalar.activation(ehqB[:], qrawB[:], EXP, scale=UNSCALE)
                # ow: assemble exp(v3h) as 2x[128e, 128d]
                ehvB = []
                for k in range(2):
                    vr = bpool.tile([DIM, DIM], BF16, name=f"vraw{k}",
                                    tag=f"vraw{k}")
                    nc.gpsimd.dma_start(out=vr[:],
                                        in_=hvview[i, 4 * k:4 * k + 4])
                    ev = bpool.tile([DIM, DIM], BF16, name=f"ehvB{k}",
                                    tag=f"ehvB{i}_{k}")
                    nc.scalar.activation(ev[:], vr[:], EXP, scale=UNSCALE)
                    ehvB.append(ev)
                # colsums: wv normalizer (per e) cols 0,1; ow S_d col 2
                pcs2 = psB.tile([DIM, 512], F32, name="pcs2", tag="pB")
                for j in range(2):
                    nc.tensor.matmul(pcs2[:, j:j + 1],
                                     ehqB[:, 128 * j:128 * (j + 1)],
                                     ones_col[:], start=True, stop=True)
                for k in range(2):
                    nc.tensor.matmul(pcs2[:, 2:3], ehvB[k][:], ones_col[:],
                                     start=(k == 0), stop=(k == 1))
                recipB = bpool.tile([DIM, 4], F32, name=f"recipB{i}",
                                    tag=f"recipB{i}")
                nc.vector.reciprocal(recipB[:, 0:3], pcs2[:, 0:3])
                # fold wv normalizer (per e-row) into ow rows
                for k in range(2):
                    nc.vector.tensor_scalar_mul(ehvB[k][:], ehvB[k][:],
                                                recipB[:, k:k + 1])
                # v = x @ exp(wv) (unnormalized; fixed via ehvB rows above)
                v_sb = []
                for mt, (m0, msz) in enumerate(MT):
                    pv = psA.tile([DIM, 1024], F32, name="pv", tag="pA")
                    nc.tensor.matmul(pv[:msz, :INNER],
                                     xT_sb[i][:, m0:m0 + msz],
                                     ehqB[:], start=True, stop=True)
                    s = bpool.tile([128, INNER], BF16, name=f"v_sb{mt}",
                                   tag=f"v_sb{i}_{mt}")
                    nc.scalar.copy(s[:msz, :], pv[:msz, :INNER])
                    v_sb.append(s)
                s_i["ehvB"], s_i["recipB"], s_i["v"] = ehvB, recipB, v_sb
                s_i["outTB"] = [bpool.tile([DIM, N], BF16, name=f"outTB{k}",
                                           tag=f"outTB{i}_{k}")
                                for k in range(2)]

            def part2_po(i, kg, hf):
                # attn@v pack: 4 heads/bank; TT applies 1/den and lands in
                # outTB[kg] (e-rows) for the y matmul.
                s_i = st_[i]
                n0, nsz = NH[hf]
                pool_, tag_ = ((psB, "pB") if (2 * kg + hf) % 2 == 0
                               else (psD, "pD"))
                po = pool_.tile([128, 512], F32, name="po", tag=tag_)
                for j in range(4):
                    h = 4 * kg + j
                    for mt, (m0, msz) in enumerate(MT):
                        nc.tensor.matmul(
                            po[32 * j:32 * j + 32, :nsz],
                            s_i["v"][mt][:msz, 32 * h:32 * h + 32],
                            s_i["em"][(h, mt)][:msz, n0:n0 + nsz],
                            start=(mt == 0), stop=(mt == 4),
                            tile_position=(0, 32 * j))
                with nc.allow_low_precision("attn out bf16"):
                    nc.vector.tensor_mul(s_i["outTB"][kg][:, n0:n0 + nsz],
                                         po[:, :nsz],
                                         s_i["rden"][(kg, hf)][:])

            def part2_y(i):
                # split by n-half so the copy/store of half 0 overlaps the
                # half-1 matmuls (shaves the kernel tail)
                s_i = st_[i]
                py = psA.tile([DIM, 1024], F32, name="py", tag="pA")
                pyv = py[:].rearrange("p (b n) -> p b n", b=2)
                ys = bpool.tile([DIM, N], BF16, name="ys", tag=f"ys{i}")
                for hf, (n0, nsz) in enumerate(NH):
                    for k in range(2):
                        nc.tensor.matmul(
                            pyv[:, hf, :nsz], s_i["ehvB"][k][:],
                            s_i["outTB"][k][:, n0:n0 + nsz],
                            start=(k == 0), stop=(k == 1))
                    nc.scalar.activation(
                        ys[:, n0:n0 + nsz], pyv[:, hf, :nsz],
                        mybir.ActivationFunctionType.Copy,
                        scale=s_i["recipB"][:, 2:3])
                    nc.sync.dma_start(out=yT[i, :, n0:n0 + nsz],
                                      in_=ys[:, n0:n0 + nsz])

            part1(0)
            part2_pre(0)

            def inject_b0(h):
                # b0's attn@v chains run in b1's part-1 PE gaps
                if h in (1, 3, 5, 7):
                    kg, hf = divmod((h - 1) // 2, 2)
                    part2_po(0, kg, hf)

            part1(1, inject=inject_b0)
            part2_y(0)
            part2_pre(1)
            for kg in range(2):
                for hf in range(2):
                    part2_po(1, kg, hf)
            part2_y(1)
            emctx.__exit__(None, None, None)
            psDctx.__exit__(None, None, None)

    return nc


_PROGRAM_CACHE = {}


def _get_program(with_bias):
    if with_bias not in _PROGRAM_CACHE:
        _PROGRAM_CACHE[with_bias] = _build_program(with_bias)
    return _PROGRAM_CACHE[with_bias]


def _shard_inputs(x, mask, resolution, framerate,
                  W1, b1, W2, b2, W3, b3, V1, c1, V2, c2, V3, c3, with_bias):
    b12, c12, b3f, c3f = with_bias
    x = np.asarray(x, np.float32)
    mask = np.asarray(mask, np.float32)
    xT = np.ascontiguousarray(x.transpose(0, 2, 1)).astype(BF16_NP)
    maskTn = np.ascontiguousarray(
        mask[0, :, 0].transpose(0, 2, 1)).astype(BF16_NP)     # [B, 600m, 600n]
    maskTp = np.zeros((B, 5 * DIM, N), BF16_NP)
    maskTp[:, :N, :] = maskTn
    maskTp = maskTp.reshape(B, 5, DIM, N)
    frateT = np.ascontiguousarray(
        np.stack([np.asarray(framerate, np.float32),
                  np.asarray(resolution, np.float32)], axis=0))
    W1 = np.ascontiguousarray(np.asarray(W1, np.float32))
    W2 = np.ascontiguousarray(np.asarray(W2, np.float32))
    V1 = np.ascontiguousarray(np.asarray(V1, np.float32))
    V2 = np.ascontiguousarray(np.asarray(V2, np.float32))
    W3v = np.asarray(W3, np.float32).reshape(D3, DIM, E3)
    V3v = np.asarray(V3, np.float32).reshape(INNER, INNER, DIM)
    in_maps = []
    for c in range(NCORES):
        m = {
            "xT": xT[BPC * c:BPC * (c + 1)],
            "maskT": maskTp[BPC * c:BPC * (c + 1)],
            "frateT": frateT,
            "w1": W1, "w2": W2, "v1": V1, "v2": V2,
            # reordered: (d-major, e<512) then (d-major, e>=512) — matches
            # the split-a2a chunk layout in the device program
            "w3c": (np.concatenate([
                W3v[:, 16 * c:16 * (c + 1), :512].reshape(D3, 16 * 512),
                W3v[:, 16 * c:16 * (c + 1), 512:].reshape(D3, 16 * 256),
            ], axis=1) * W3_SCALE).astype(FP8_NP),
            "v3c": (np.ascontiguousarray(
                V3v[:, 32 * c:32 * (c + 1), :]).reshape(INNER, V3_SL)
                * W3_SCALE).astype(FP8_NP),
        }
        if b12:
            m["b1t"] = np.asarray(b1, np.float32).reshape(D3, 1)
            m["b2t"] = np.asarray(b2, np.float32).reshape(D3, 1)
        if c12:
            m["c1t"] = np.asarray(c1, np.float32).reshape(INNER, 1)
            m["c2t"] = np.asarray(c2, np.float32).reshape(INNER, 1)
        if b3f:
            b3v = np.asarray(b3, np.float32).reshape(DIM, E3)[16 * c:16 * (c + 1)]
            m["b3c"] = (np.concatenate(
                [b3v[:, :512].reshape(1, 16 * 512),
                 b3v[:, 512:].reshape(1, 16 * 256)], axis=1)
                * (W3_SCALE * A2_SCALE)).astype(BF16_NP)
        if c3f:
            m["c3c"] = (np.ascontiguousarray(
                np.asarray(c3, np.float32).reshape(INNER, DIM)
                [32 * c:32 * (c + 1)].reshape(1, V3_SL))
                * (W3_SCALE * A2_SCALE)).astype(BF16_NP)
        in_maps.append(m)
    return in_maps


def _run(inputs, trace=False, tmpdir=None):
    with_bias = (
        bool(np.any(inputs["b1"])) or bool(np.any(inputs["b2"])),
        bool(np.any(inputs["c1"])) or bool(np.any(inputs["c2"])),
        bool(np.any(inputs["b3"])),
        bool(np.any(inputs["c3"])),
    )
    nc = _get_program(with_bias)
    in_maps = _shard_inputs(with_bias=with_bias, **inputs)
    res = run_bass_kernel_spmd(nc, in_maps, core_ids=list(range(NCORES)),
                               trace=trace, tmpdir=tmpdir)
    outs = []
    for c in range(NCORES):
        yt = np.asarray(res.results[c]["yT"], dtype=np.float32)  # [2,128,600]
        outs.append(yt.transpose(0, 2, 1))  # [2, 600, 128]
    full = np.ascontiguousarray(np.concatenate(outs, axis=0)).astype(np.float32)
    return full, res


def kernel(**inputs) -> np.ndarray:
    out, _ = _run(inputs, trace=False)
    return out


# revision 34
# speedup vs baseline: 1.0668x; 1.0668x over previous
"""Trainium2 Bass kernel for nn_Attention_86423331930617.

Reference math (per batch b of 16):
  frate = [framerate[b], resolution[b]]                       # [2]
  h  = ((frate@W1+b1)@W2+b2)@W3+b3                            # [98304]
  qkvw = softmax(h.reshape(128, 768), axis=0)                 # over dim d
  wq, wk, wv = split(qkvw, 3, -1)                             # [128, 256] each
  q/k/v = x[b] @ w*      -> heads [8, 600, 32]
  dots = q@k.T / sqrt(32); attn = softmax(dots, -1) * mask[b]
  out  = attn @ v -> [600, 256]
  ow   = softmax(((frate@V1+c1)@V2+c2)@V3+c3 .reshape(256,128), axis=0)
  y    = out @ ow                                             # [600, 128]

Distribution over 8 cores (single NEFF, three AllToAlls):
  - warmup 256B AllToAll at t~0 absorbs the collective rendezvous
    barrier while the hypernet computes.
  - Hypernet big matmuls column-sharded (fp8 DoubleRow ks-pairs);
    A2A #1 ships q/k right after the 16 qk chunks; A2A #2 ships wv + ow
    and overlaps attention part 1.
  - Attention batch-sharded: core c does batches 2c, 2c+1.
DMA discipline: hypernet stores issue on the gpsimd queue (in front of
the collectives they feed), x/mask loads on the vector queue, weights +
a2a unpacks on sync. Unpacks/mask are single strided DMAs.
Packing tricks (tile_position inferred from out-AP base partition):
  - hypernet h-chunks: 4x [16,512] packed per PSUM bank -> 1 ACT copy
  - rowsums: ones[m,32] stationary -> [32,n] broadcast, 4 heads/bank
  - attn out (po): 4 heads/bank -> single TT applies 1/den and lands
    directly in the y-matmul operand layout
Softmax normalizers fold into ACT scales; exp needs no max-subtraction
(hypernet outputs and dots are O(0.1) by construction).
"""
import sys

sys.path.insert(0, "/opt/trn_rl_repo")
if "/root/.axon_site" not in sys.path:
    sys.path.insert(0, "/root/.axon_site")

import numpy as np
import ml_dtypes

import concourse.bass as bass
import concourse.mybir as mybir
import concourse.tile as tile
from concourse.vector_clock import ScopedClock
from concourse.bass_utils import run_bass_kernel_spmd

F32 = mybir.dt.float32
BF16 = mybir.dt.bfloat16
BF16_NP = ml_dtypes.bfloat16
FP8 = mybir.dt.float8e4
FP8_NP = mybir.dt.np(mybir.dt.float8e4)
DR = mybir.MatmulPerfMode.DoubleRow
W3_SCALE = 64.0
A2_SCALE = 4.0
UNSCALE = 1.0 / (W3_SCALE * A2_SCALE)
EXP = mybir.ActivationFunctionType.Exp
IDENT = mybir.ActivationFunctionType.Identity
MULT = mybir.AluOpType.mult

NCORES = 8
B, N, DIM, HEADS, DH = 16, 600, 128, 8, 32
INNER = HEADS * DH          # 256
D3 = 3 * DIM                # 384
E3 = 3 * INNER              # 768
BPC = B // NCORES           # batches per core = 2
W3_SL = 16 * E3             # 12288 w3 cols per core (16 d-rows)
V3_SL = 32 * DIM            # 4096 v3 cols per core (32 i-rows)
SCALE = DH ** -0.5
NH = [(0, 300), (300, 300)]                       # n halves
MT = [(0, 128), (128, 128), (256, 128), (384, 128), (512, 88)]  # m tiles
CH = 512                    # hypernet chunk cols
QK_W = 16 * 512             # 8192: q/k cols per core slice (a2a #1)
VB_W = 16 * 256             # 4096: wv cols per core slice (a2a #2)
A2_W = VB_W + V3_SL         # 8192: a2a #2 width (wv + ow)


# ---------------------------------------------------------------------------
# This walrus build accepts at most ONE sync wait / update per instruction;
# Tile can emit more. Split extras onto adjacent same-engine NoOps.
class _SplitWaitTileContext(tile.TileContext):
    def _split_sync(self, insts):
        out = []
        for inst in insts:
            si = inst.sync_info
            if si is None:
                out.append(inst)
                continue
            waits = list(si.on_wait) if si.on_wait else []
            updates = list(si.on_update) if si.on_update else []
            if len(waits) <= 1 and len(updates) <= 1:
                out.append(inst)
                continue
            for w in waits[1:]:
                nop = mybir.InstNoOp(name=f"I-{self.nc.next_id()}", ins=[], outs=[])
                nop.engine = inst.engine
                nop.sync_info = mybir.SyncInfo(on_wait=[w], on_update=[])
                out.append(nop)
            inst.sync_info = mybir.SyncInfo(on_wait=waits[:1], on_update=updates[:1])
            out.append(inst)
            for u in updates[1:]:
                nop = mybir.InstNoOp(name=f"I-{self.nc.next_id()}", ins=[], outs=[])
                nop.engine = inst.engine
                nop.sync_info = mybir.SyncInfo(on_wait=[], on_update=[u])
                out.append(nop)
        return out

    def _lower_ordered_insts(self, ordered):
        for bb_name in list(ordered.keys()):
            ordered[bb_name] = self._split_sync(ordered[bb_name])
        return super()._lower_ordered_insts(ordered)

    def _drain_and_barrier(self, tick_clock, wait_clock):
        nc = self.nc
        probe = nc.sync.nop()
        wait_clock.add_sem_waits(probe.ins, ScopedClock({None: tick_clock.global_clock}))
        si = probe.ins.sync_info
        waits = list(si.on_wait) if si is not None and si.on_wait else []
        if len(waits) > 1:
            probe.ins.sync_info = mybir.SyncInfo(on_wait=waits[:1], on_update=[])
            for w in waits[1:]:
                extra = nc.sync.nop()
                extra.ins.sync_info = mybir.SyncInfo(on_wait=[w], on_update=[])
        nc.sync.drain()
        nc.all_engine_barrier()
        assert self.sems is not None
        popped = nc._tile_sem_poison_stack.pop()
        assert popped is self._sem_poison
        nc.clear_and_free_semaphores(list(self.sems.allocated().values()))
        nc.all_engine_barrier()


# ---------------------------------------------------------------------------
def _build_program(with_bias):
    """Emit the per-core SPMD program. with_bias: (b12, c12, b3, c3) flags."""
    b12, c12, b3f, c3f = with_bias
    nc = bass.Bass("TRN2", target_bir_lowering=False, debug=False,
                   num_devices=NCORES)

    xT = nc.dram_tensor("xT", [BPC, DIM, N], BF16, kind="ExternalInput")
    maskT = nc.dram_tensor("maskT", [BPC, 5, DIM, N], BF16, kind="ExternalInput")
    frateT = nc.dram_tensor("frateT", [2, B], F32, kind="ExternalInput")
    w1 = nc.dram_tensor("w1", [2, D3], F32, kind="ExternalInput")
    w2 = nc.dram_tensor("w2", [D3, D3], F32, kind="ExternalInput")
    w3c = nc.dram_tensor("w3c", [D3, W3_SL], FP8, kind="ExternalInput")
    v1 = nc.dram_tensor("v1", [2, INNER], F32, kind="ExternalInput")
    v2 = nc.dram_tensor("v2", [INNER, INNER], F32, kind="ExternalInput")
    v3c = nc.dram_tensor("v3c", [INNER, V3_SL], FP8, kind="ExternalInput")
    if b12:
        b1t = nc.dram_tensor("b1t", [D3, 1], F32, kind="ExternalInput")
        b2t = nc.dram_tensor("b2t", [D3, 1], F32, kind="ExternalInput")
    if c12:
        c1t = nc.dram_tensor("c1t", [INNER, 1], F32, kind="ExternalInput")
        c2t = nc.dram_tensor("c2t", [INNER, 1], F32, kind="ExternalInput")
    if b3f:
        b3c = nc.dram_tensor("b3c", [1, W3_SL], BF16, kind="ExternalInput")
    if c3f:
        c3c = nc.dram_tensor("c3c", [1, V3_SL], BF16, kind="ExternalInput")
    yT = nc.dram_tensor("yT", [BPC, DIM, N], BF16, kind="ExternalOutput")

    with _SplitWaitTileContext(nc) as tc:
        with (
            tc.tile_pool(name="const", bufs=1) as cpool,
            tc.tile_pool(name="wts", bufs=1) as wpool,
            tc.tile_pool(name="achain", bufs=1) as apool,
            tc.tile_pool(name="hcopy", bufs=6) as hpool,
            tc.tile_pool(name="batch", bufs=1) as bpool,
            tc.tile_pool(name="etile", bufs=30) as epool,
            tc.tile_pool(name="rden", bufs=8) as rpool,
            # PSUM budget (8 banks of 2KB/partition):
            #   psA 3x2 banks (dots/a-chain/proj/y), psH 1 (hypernet chunk
            #   strips; ctx-freed after phase A for psD), psB 1 (rowsum
            #   packs, po packs, colsums; ping-pongs with psD in attention)
            tc.tile_pool(name="psA", bufs=3, space="PSUM") as psA,
            tc.tile_pool(name="psB", bufs=1, space="PSUM") as psB,
            tc.tile_pool(name="dram", bufs=1, space="DRAM") as dpool,
        ):
            # ---- constants
            ones_col = cpool.tile([DIM, 1], BF16, name="ones_col")
            nc.vector.memset(ones_col[:], 1.0)
            ones32 = cpool.tile([DIM, 32], BF16, name="ones32")
            nc.vector.memset(ones32[:], 1.0)
            if b3f or c3f:
                ones_row16 = cpool.tile([1, B], BF16, name="ones_row16")
                nc.vector.memset(ones_row16[:], 1.0)

            # ---- small weights in (sync queue)
            fr_sb = wpool.tile([2, B], F32, name="fr_sb")
            nc.sync.dma_start(out=fr_sb[:], in_=frateT[:])
            w1_sb = wpool.tile([2, D3], F32, name="w1_sb")
            nc.sync.dma_start(out=w1_sb[:], in_=w1[:])
            w2_sb = wpool.tile([DIM, 3, D3], F32, name="w2_sb")
            nc.sync.dma_start(out=w2_sb[:],
                              in_=w2[:].rearrange("(k p) n -> p k n", p=DIM))
            v1_sb = wpool.tile([2, INNER], F32, name="v1_sb")
            nc.sync.dma_start(out=v1_sb[:], in_=v1[:])
            v2_sb = wpool.tile([DIM, 2, INNER], F32, name="v2_sb")
            nc.sync.dma_start(out=v2_sb[:],
                              in_=v2[:].rearrange("(k p) n -> p k n", p=DIM))
            bias_sb = {}
            if b12:
                bias_sb["b1"] = wpool.tile([D3, 1], F32, name="b1_sb")
                nc.sync.dma_start(out=bias_sb["b1"][:], in_=b1t[:])
                bias_sb["b2"] = wpool.tile([D3, 1], F32, name="b2_sb")
                nc.sync.dma_start(out=bias_sb["b2"][:], in_=b2t[:])
            if c12:
                bias_sb["c1"] = wpool.tile([INNER, 1], F32, name="c1_sb")
                nc.sync.dma_start(out=bias_sb["c1"][:], in_=c1t[:])
                bias_sb["c2"] = wpool.tile([INNER, 1], F32, name="c2_sb")
                nc.sync.dma_start(out=bias_sb["c2"][:], in_=c2t[:])
            if b3f:
                b3_sb = wpool.tile([1, W3_SL], BF16, name="b3_sb")
                nc.sync.dma_start(out=b3_sb[:], in_=b3c[:])
            if c3f:
                c3_sb = wpool.tile([1, V3_SL], BF16, name="c3_sb")
                nc.sync.dma_start(out=c3_sb[:], in_=c3c[:])

            # ---- big hypernet weight slices (freed before phase B needs
            # the masked-exp tile pool)
            w3ctx = tc.tile_pool(name="w3", bufs=1)
            w3pool = w3ctx.__enter__()
            w3dr = w3pool.tile([DIM, 3, W3_SL], FP8, name="w3dr")
            w3view = w3c[:].rearrange("(ks p) n -> p ks n", p=DIM)
            for q4 in range(4):
                q0 = q4 * (W3_SL // 4)
                nc.sync.dma_start(out=w3dr[:, :, q0:q0 + W3_SL // 4],
                                  in_=w3view[:, :, q0:q0 + W3_SL // 4])
            v3dr = w3pool.tile([DIM, 2, V3_SL], FP8, name="v3dr")
            nc.sync.dma_start(
                out=v3dr[:], in_=v3c[:].rearrange("(ks p) n -> p ks n", p=DIM))

            # ---- a-chain: a1T = (frate@W1+b1).T as 3x[128,16]
            a1T = []
            for t in range(3):
                p = psA.tile([DIM, 1024], F32, name="pa", tag="pA")
                nc.tensor.matmul(p[:, :B], w1_sb[:, 128 * t:128 * (t + 1)],
                                 fr_sb[:], start=True, stop=True)
                s = apool.tile([DIM, B], F32, name=f"a1T{t}")
                if b12:
                    nc.scalar.activation(s[:], p[:, :B], IDENT,
                                         bias=bias_sb["b1"][128 * t:128 * (t + 1), :])
                else:
                    nc.scalar.copy(s[:], p[:, :B])
                a1T.append(s)
            a2f8 = apool.tile([DIM, 3, B], FP8, name="a2f8")
            for t in range(3):
                p = psA.tile([DIM, 1024], F32, name="pa2", tag="pA")
                for k in range(3):
                    nc.tensor.matmul(p[:, :B], w2_sb[:, k, 128 * t:128 * (t + 1)],
                                     a1T[k][:], start=(k == 0), stop=(k == 2))
                if b12:
                    tmp = apool.tile([DIM, B], F32, name=f"a2tmp{t}")
                    nc.scalar.activation(tmp[:], p[:, :B], IDENT,
                                         bias=bias_sb["b2"][128 * t:128 * (t + 1), :])
                    with nc.allow_low_precision("fp8 hypernet activations"):
                        nc.vector.tensor_scalar_mul(a2f8[:, t, :], tmp[:],
                                                    A2_SCALE)
                else:
                    nc.scalar.mul(a2f8[:, t, :], p[:, :B], A2_SCALE)
            av1T = []
            for t in range(2):
                p = psA.tile([DIM, 1024], F32, name="pav", tag="pA")
                nc.tensor.matmul(p[:, :B], v1_sb[:, 128 * t:128 * (t + 1)],
                                 fr_sb[:], start=True, stop=True)
                s = apool.tile([DIM, B], F32, name=f"av1T{t}")
                if c12:
                    nc.scalar.activation(s[:], p[:, :B], IDENT,
                                         bias=bias_sb["c1"][128 * t:128 * (t + 1), :])
                else:
                    nc.scalar.copy(s[:], p[:, :B])
                av1T.append(s)
            avf8 = apool.tile([DIM, 2, B], FP8, name="avf8")
            for t in range(2):
                p = psA.tile([DIM, 1024], F32, name="pav2", tag="pA")
                for k in range(2):
                    nc.tensor.matmul(p[:, :B], v2_sb[:, k, 128 * t:128 * (t + 1)],
                                     av1T[k][:], start=(k == 0), stop=(k == 1))
                if c12:
                    tmp = apool.tile([DIM, B], F32, name=f"avtmp{t}")
                    nc.scalar.activation(tmp[:], p[:, :B], IDENT,
                                         bias=bias_sb["c2"][128 * t:128 * (t + 1), :])
                    with nc.allow_low_precision("fp8 hypernet activations"):
                        nc.vector.tensor_scalar_mul(avf8[:, t, :], tmp[:],
                                                    A2_SCALE)
                else:
                    nc.scalar.mul(avf8[:, t, :], p[:, :B], A2_SCALE)

            # ---- x / mask inputs on the scalar queue, emitted after the
            # a-chain so its ACT ops aren't queued behind DMA dispatches
            xT_sb = [bpool.tile([DIM, N], BF16, name=f"xT_sb{i}")
                     for i in range(BPC)]
            for i in range(BPC):
                nc.scalar.dma_start(out=xT_sb[i][:], in_=xT[i])
            maskT_sb = [bpool.tile([DIM, 5, N], BF16, name=f"mask_sb{i}")
                        for i in range(BPC)]
            for i in range(BPC):
                nc.scalar.dma_start(
                    out=maskT_sb[i][:],
                    in_=maskT[i].rearrange("mt p n -> p mt n"))

            # ---- big hypernet matmuls -> two a2a inputs
            # host reorders w3c cols: first 16x512 (d-major, e<512 = q,k),
            # then 16x256 (e>=512 = wv). a2a #1 ships q/k right after the
            # qk chunks; a2a #2 ships wv + ow and overlaps attention part 1.
            # Chunks packed 4-per-bank at partition strips {0,32,64,96};
            # one ACT copy per 4 chunks. Output stays scaled by 1/UNSCALE;
            # consumers fold UNSCALE into their exp() scale. Stores issue on
            # the gpsimd queue in front of the collectives they feed.
            a2a1_in = dpool.tile([B, QK_W], BF16, name="a2a1_in")
            a2a2_in = dpool.tile([B, A2_W], BF16, name="a2a2_in")
            psHctx = tc.tile_pool(name="psH", bufs=1, space="PSUM")
            psH = psHctx.__enter__()

            def w3_group(g, tgt, toff, n_chunks=4):
                """Chunks 4g..4g+n_chunks of the 24 w3 chunks (512 cols)."""
                p = psH.tile([128, CH], F32, name="ph", tag="pH")
                for c in range(n_chunks):
                    j = 4 * g + c
                    st = 32 * c
                    for ks in range(3):
                        nc.tensor.matmul(p[st:st + B, :], a2f8[:, ks, :],
                                         w3dr[:, ks, CH * j:CH * (j + 1)],
                                         start=(ks == 0),
                                         stop=(ks == 2 and not b3f),
                                         tile_position=(0, st))
                    if b3f:
                        nc.tensor.matmul(p[st:st + B, :], ones_row16[:],
                                         b3_sb[:, CH * j:CH * (j + 1)],
                                         start=False, stop=True,
                                         tile_position=(0, st))
                s = hpool.tile([128, CH], BF16, name="hs", tag="hs")
                nc.scalar.copy(s[:], p[:])
                for c in range(n_chunks):
                    j = 4 * g + c
                    nc.gpsimd.dma_start(
                        out=tgt[:, CH * j - toff:CH * (j + 1) - toff],
                        in_=s[32 * c:32 * c + B, :])

            for g in range(4):          # qk chunks 0..15
                w3_group(g, a2a1_in, 0)
            a2a1_out = dpool.tile([B, QK_W], BF16, name="a2a1_out")
            nc.gpsimd.collective_compute(
                "AllToAll", mybir.AluOpType.bypass,
                replica_groups=[list(range(NCORES))],
                ins=[a2a1_in[:]], outs=[a2a1_out[:]],
            )
            for g in range(4, 6):       # wv chunks 16..23 -> a2a2 cols 0..4096
                w3_group(g, a2a2_in, QK_W)
            for g in range(2):          # v3: 8 chunks of 512 -> a2a #2 (ow)
                p = psH.tile([128, CH], F32, name="phv", tag="pH")
                for c in range(4):
                    j = 4 * g + c
                    st = 32 * c
                    for ks in range(2):
                        nc.tensor.matmul(p[st:st + B, :], avf8[:, ks, :],
                                         v3dr[:, ks, CH * j:CH * (j + 1)],
                                         start=(ks == 0),
                                         stop=(ks == 1 and not c3f),
                                         tile_position=(0, st))
                    if c3f:
                        nc.tensor.matmul(p[st:st + B, :], ones_row16[:],
                                         c3_sb[:, CH * j:CH * (j + 1)],
                                         start=False, stop=True,
                                         tile_position=(0, st))
                s = hpool.tile([128, CH], BF16, name="hvs", tag="hs")
                nc.scalar.copy(s[:], p[:])
                for c in range(4):
                    j = 4 * g + c
                    nc.gpsimd.dma_start(
                        out=a2a2_in[:, VB_W + CH * j:VB_W + CH * (j + 1)],
                        in_=s[32 * c:32 * c + B, :])
            psHctx.__exit__(None, None, None)
            # psH's bank is free once phase A drains; attention ping-pongs
            # rowsum/po packs between psB and psD so a pack's matmuls never
            # wait on the previous pack's DVE drain.
            psDctx = tc.tile_pool(name="psD", bufs=1, space="PSUM")
            psD = psDctx.__enter__()
            a2a2_out = dpool.tile([B, A2_W], BF16, name="a2a2_out")
            nc.gpsimd.collective_compute(
                "AllToAll", mybir.AluOpType.bypass,
                replica_groups=[list(range(NCORES))],
                ins=[a2a2_in[:]], outs=[a2a2_out[:]],
            )
            w3ctx.__exit__(None, None, None)
            emctx = tc.tile_pool(name="emk", bufs=62)
            em_pool = emctx.__enter__()
            # row (2s+i) holds my batch i's hypernet outputs from source s
            h1view = a2a1_out[:].rearrange(
                "(s two) (d e) -> two s d e", two=BPC, d=16)
            h2view = a2a2_out[:, :VB_W].rearrange(
                "(s two) (d e) -> two s d e", two=BPC, d=16)
            hvview = a2a2_out[:, VB_W:].rearrange(
                "(s two) (iv dd) -> two s iv dd", two=BPC, iv=32)

            # ================= attention =================
            # part1(b0) -> part2-pre(b0) -> part1(b1) with b0's attn@v
            # chains injected between heads (fills PE gaps while ACT paces
            # the exps) -> y(b0) -> part2(b1).
            st_ = [dict() for _ in range(BPC)]

            def part1(i, inject=None):
                s_i = st_[i]
                # lead-in pipelined by e-half: the q half (packs 0,1) flows
                # dma->exp->colsum->recip->proj before the k half's exp.
                qrawA = bpool.tile([DIM, 512], BF16, name="qrawA", tag="qrawA")
                for eh in range(2):
                    nc.gpsimd.dma_start(
                        out=qrawA[:, 256 * eh:256 * (eh + 1)],
                        in_=h1view[i][:, :, 256 * eh:256 * (eh + 1)])
                ehqA = bpool.tile([DIM, 512], BF16, name=f"ehqA{i}",
                                  tag=f"ehqA{i}")
                pcs = psB.tile([DIM, 512], F32, name="pcs", tag="pB")
                # recipA col p = softmax denom recip for e in [128p,128p+128)
                # == per-partition scale for qkT pack p (strips match).
                recipA = bpool.tile([DIM, 4], F32, name="recipA", tag="recipA")
                qkT = [None] * 4

                def lead_half(eh):
                    nc.scalar.activation(ehqA[:, 256 * eh:256 * (eh + 1)],
                                         qrawA[:, 256 * eh:256 * (eh + 1)],
                                         EXP, scale=UNSCALE)
                    for j in (2 * eh, 2 * eh + 1):
                        nc.tensor.matmul(pcs[:, j:j + 1],
                                         ehqA[:, 128 * j:128 * (j + 1)],
                                         ones_col[:], start=True, stop=True)
                    nc.vector.reciprocal(recipA[:, 2 * eh:2 * eh + 2],
                                         pcs[:, 2 * eh:2 * eh + 2])
                    if eh == 0:
                        nc.vector.tensor_scalar_mul(recipA[:, 0:2],
                                                    recipA[:, 0:2], SCALE)

                def proj_pack(pk):
                    pp = psA.tile([DIM, 1024], F32, name="pproj", tag="pA")
                    ppv = pp[:].rearrange("p (b n) -> p b n", b=2)
                    for st in range(2):
                        blk = 2 * pk + st
                        for hf, (n0, nsz) in enumerate(NH):
                            nc.tensor.matmul(
                                ppv[64 * st:64 * st + 64, hf, :nsz],
                                ehqA[:, 64 * blk:64 * blk + 64],
                                xT_sb[i][:, n0:n0 + nsz],
                                start=True, stop=True,
                                tile_position=(0, 64 * st))
                    s = bpool.tile([DIM, N], BF16, name=f"qkT{pk}",
                                   tag=f"qkT{i}_{pk}")
                    sview = s[:].rearrange("p (hf n) -> p hf n", hf=2)
                    nc.scalar.activation(
                        sview,
                        pp[:].rearrange("p (hf n) -> p hf n", hf=2)[:, :, :300],
                        mybir.ActivationFunctionType.Copy,
                        scale=recipA[:, pk:pk + 1])
                    qkT[pk] = s

                # heads 0-3 consume packs (0, 2): emit those first
                lead_half(0)
                proj_pack(0)
                lead_half(1)
                proj_pack(2)
                proj_pack(1)
                proj_pack(3)
                em_all, em_eun, rden_all = {}, {}, {}
                s_i["em"], s_i["rden"] = em_all, rden_all

                def emit_rowsum_burst(kg):
                    # 20 contiguous full-K PE matmuls per half
                    for hf, (n0, nsz) in enumerate(NH):
                        pool_, tag_ = ((psB, "pB") if (2 * kg + hf) % 2 == 0
                                       else (psD, "pD"))
                        prs = pool_.tile([128, 512], F32, name="prs", tag=tag_)
                        for j in range(4):
                            h = 4 * kg + j
                            for mt, (m0, msz) in enumerate(MT):
                                nc.tensor.matmul(
                                    prs[32 * j:32 * j + 32, :nsz],
                                    ones32[:msz, :],
                                    em_eun[(h, mt)][:msz, n0:n0 + nsz],
                                    start=(mt == 0), stop=(mt == 4),
                                    tile_position=(0, 32 * j))
                        rden = rpool.tile([128, 300], F32, name="rden",
                                          tag="rden")
                        nc.vector.reciprocal(rden[:], prs[:, :300])
                        rden_all[(kg, hf)] = rden

                for h in range(HEADS):
                    pk, ro = h // 4, 32 * (h % 4)
                    qp, kp = qkT[pk], qkT[2 + pk]
                    for mt, (m0, msz) in enumerate(MT):
                        pd = psA.tile([128, 1024], F32, name="pdots", tag="pA")
                        pdv = pd[:].rearrange("p (b n) -> p b n", b=2)
                        for hf, (n0, nsz) in enumerate(NH):
                            nc.tensor.matmul(
                                pdv[:msz, hf, :nsz],
                                kp[ro:ro + 32, m0:m0 + msz],
                                qp[ro:ro + 32, n0:n0 + nsz],
                                start=True, stop=True,
                                tile_position=(ro, 0))
                        e_t = epool.tile([128, N], BF16, name="e_t", tag="e")
                        nc.scalar.activation(
                            e_t[:msz].rearrange("p (hf n) -> p hf n", hf=2),
                            pd[:msz].rearrange(
                                "p (hf n) -> p hf n", hf=2)[:, :, :300],
                            EXP)
                        em_t = em_pool.tile([128, N], BF16, name="em_t",
                                            tag="em")
                        nc.vector.tensor_mul(em_t[:msz], e_t[:msz],
                                             maskT_sb[i][:msz, mt, :])
                        em_eun[(h, mt)] = e_t
                        em_all[(h, mt)] = em_t
                    if h == 4:
                        emit_rowsum_burst(0)
                    if inject is not None:
                        inject(h)
                emit_rowsum_burst(1)

            def part2_pre(i):
                s_i = st_[i]
                qrawB = bpool.tile([DIM, 256], BF16, name="qrawB", tag="qrawB")
                nc.gpsimd.dma_start(out=qrawB[:], in_=h2view[i])
                ehqB = bpool.tile([DIM, 256], BF16, name=f"ehqB{i}",
                                  tag=f"ehqB{i}")
                nc.scalar.activation(ehqB[:], qrawB[:], EXP, scale=UNSCALE)
                # ow: assemble exp(v3h) as 2x[128e, 128d]
                ehvB = []
                for k in range(2):
                    vr = bpool.tile([DIM, DIM], BF16, name=f"vraw{k}",
                                    tag=f"vraw{k}")
                    nc.gpsimd.dma_start(out=vr[:],
                                        in_=hvview[i, 4 * k:4 * k + 4])
                    ev = bpool.tile([DIM, DIM], BF16, name=f"ehvB{k}",
                                    tag=f"ehvB{i}_{k}")
                    nc.scalar.activation(ev[:], vr[:], EXP, scale=UNSCALE)
                    ehvB.append(ev)
                # colsums: wv normalizer (per e) cols 0,1; ow S_d col 2
                pcs2 = psB.tile([DIM, 512], F32, name="pcs2", tag="pB")
                for j in range(2):
                    nc.tensor.matmul(pcs2[:, j:j + 1],
                                     ehqB[:, 128 * j:128 * (j + 1)],
                                     ones_col[:], start=True, stop=True)
                for k in range(2):
                    nc.tensor.matmul(pcs2[:, 2:3], ehvB[k][:], ones_col[:],
                                     start=(k == 0), stop=(k == 1))
                recipB = bpool.tile([DIM, 4], F32, name=f"recipB{i}",
                                    tag=f"recipB{i}")
                nc.vector.reciprocal(recipB[:, 0:3], pcs2[:, 0:3])
                # fold wv normalizer (per e-row) into ow rows
                for k in range(2):
                    nc.vector.tensor_scalar_mul(ehvB[k][:], ehvB[k][:],
                                                recipB[:, k:k + 1])
                # v = x @ exp(wv) (unnormalized; fixed via ehvB rows above)
                v_sb = []
                for mt, (m0, msz) in enumerate(MT):
                    pv = psA.tile([DIM, 1024], F32, name="pv", tag="pA")
                    nc.tensor.matmul(pv[:msz, :INNER],
                                     xT_sb[i][:, m0:m0 + msz],
                                     ehqB[:], start=True, stop=True)
                    s = bpool.tile([128, INNER], BF16, name=f"v_sb{mt}",
                                   tag=f"v_sb{i}_{mt}")
                    nc.scalar.copy(s[:msz, :], pv[:msz, :INNER])
                    v_sb.append(s)
                s_i["ehvB"], s_i["recipB"], s_i["v"] = ehvB, recipB, v_sb
                s_i["outTB"] = [bpool.tile([DIM, N], BF16, name=f"outTB{k}",
                                           tag=f"outTB{i}_{k}")
                                for k in range(2)]

            def part2_po(i, kg, hf):
                # attn@v pack: 4 heads/bank; TT applies 1/den and lands in
                # outTB[kg] (e-rows) for the y matmul.
                s_i = st_[i]
                n0, nsz = NH[hf]
                pool_, tag_ = ((psB, "pB") if (2 * kg + hf) % 2 == 0
                               else (psD, "pD"))
                po = pool_.tile([128, 512], F32, name="po", tag=tag_)
                for j in range(4):
                    h = 4 * kg + j
                    for mt, (m0, msz) in enumerate(MT):
                        nc.tensor.matmul(
                            po[32 * j:32 * j + 32, :nsz],
                            s_i["v"][mt][:msz, 32 * h:32 * h + 32],
                            s_i["em"][(h, mt)][:msz, n0:n0 + nsz],
                            start=(mt == 0), stop=(mt == 4),
                            tile_position=(0, 32 * j))
                with nc.allow_low_precision("attn out bf16"):
                    nc.vector.tensor_mul(s_i["outTB"][kg][:, n0:n0 + nsz],
                                         po[:, :nsz],
                                         s_i["rden"][(kg, hf)][:])

            def part2_y(i):
                # split by n-half so the copy/store of half 0 overlaps the
                # half-1 matmuls (shaves the kernel tail)
                s_i = st_[i]
                py = psA.tile([DIM, 1024], F32, name="py", tag="pA")
                pyv = py[:].rearrange("p (b n) -> p b n", b=2)
                ys = bpool.tile([DIM, N], BF16, name="ys", tag=f"ys{i}")
                for hf, (n0, nsz) in enumerate(NH):
                    for k in range(2):
                        nc.tensor.matmul(
                            pyv[:, hf, :nsz], s_i["ehvB"][k][:],
                            s_i["outTB"][k][:, n0:n0 + nsz],
                            start=(k == 0), stop=(k == 1))
                    nc.scalar.activation(
                        ys[:, n0:n0 + nsz], pyv[:, hf, :nsz],
                        mybir.ActivationFunctionType.Copy,
                        scale=s_i["recipB"][:, 2:3])
                    nc.sync.dma_start(out=yT[i, :, n0:n0 + nsz],
                                      in_=ys[:, n0:n0 + nsz])

            part1(0)
            part2_pre(0)

            def inject_b0(h):
                # b0's attn@v chains run in b1's part-1 PE gaps
                if h in (1, 3, 5, 7):
                    kg, hf = divmod((h - 1) // 2, 2)
                    part2_po(0, kg, hf)

            part1(1, inject=inject_b0)
            part2_y(0)
            part2_pre(1)
            for kg in range(2):
                for hf in range(2):
                    part2_po(1, kg, hf)
            part2_y(1)
            emctx.__exit__(None, None, None)
            psDctx.__exit__(None, None, None)

    return nc


_PROGRAM_CACHE = {}


def _get_program(with_bias):
    if with_bias not in _PROGRAM_CACHE:
        _PROGRAM_CACHE[with_bias] = _build_program(with_bias)
    return _PROGRAM_CACHE[with_bias]


def _shard_inputs(x, mask, resolution, framerate,
                  W1, b1, W2, b2, W3, b3, V1, c1, V2, c2, V3, c3, with_bias):
    b12, c12, b3f, c3f = with_bias
    x = np.asarray(x, np.float32)
    mask = np.asarray(mask, np.float32)
    xT = np.ascontiguousarray(x.transpose(0, 2, 1)).astype(BF16_NP)
    maskTn = np.ascontiguousarray(
        mask[0, :, 0].transpose(0, 2, 1)).astype(BF16_NP)     # [B, 600m, 600n]
    maskTp = np.zeros((B, 5 * DIM, N), BF16_NP)
    maskTp[:, :N, :] = maskTn
    maskTp = maskTp.reshape(B, 5, DIM, N)
    frateT = np.ascontiguousarray(
        np.stack([np.asarray(framerate, np.float32),
                  np.asarray(resolution, np.float32)], axis=0))
    W1 = np.ascontiguousarray(np.asarray(W1, np.float32))
    W2 = np.ascontiguousarray(np.asarray(W2, np.float32))
    V1 = np.ascontiguousarray(np.asarray(V1, np.float32))
    V2 = np.ascontiguousarray(np.asarray(V2, np.float32))
    W3v = np.asarray(W3, np.float32).reshape(D3, DIM, E3)
    V3v = np.asarray(V3, np.float32).reshape(INNER, INNER, DIM)
    in_maps = []
    for c in range(NCORES):
        m = {
            "xT": xT[BPC * c:BPC * (c + 1)],
            "maskT": maskTp[BPC * c:BPC * (c + 1)],
            "frateT": frateT,
            "w1": W1, "w2": W2, "v1": V1, "v2": V2,
            # reordered: (d-major, e<512) then (d-major, e>=512) — matches
            # the split-a2a chunk layout in the device program
            "w3c": (np.concatenate([
                W3v[:, 16 * c:16 * (c + 1), :512].reshape(D3, 16 * 512),
                W3v[:, 16 * c:16 * (c + 1), 512:].reshape(D3, 16 * 256),
            ], axis=1) * W3_SCALE).astype(FP8_NP),
            "v3c": (np.ascontiguousarray(
                V3v[:, 32 * c:32 * (c + 1), :]).reshape(INNER, V3_SL)
                * W3_SCALE).astype(FP8_NP),
        }
        if b12:
            m["b1t"] = np.asarray(b1, np.float32).reshape(D3, 1)
            m["b2t"] = np.asarray(b2, np.float32).reshape(D3, 1)
        if c12:
            m["c1t"] = np.asarray(c1, np.float32).reshape(INNER, 1)
            m["c2t"] = np.asarray(c2, np.float32).reshape(INNER, 1)
        if b3f:
            b3v = np.asarray(b3, np.float32).reshape(DIM, E3)[16 * c:16 * (c + 1)]
            m["b3c"] = (np.concatenate(
                [b3v[:, :512].reshape(1, 16 * 512),
                 b3v[:, 512:].reshape(1, 16 * 256)], axis=1)
                * (W3_SCALE * A2_SCALE)).astype(BF16_NP)
        if c3f:
            m["c3c"] = (np.ascontiguousarray(
                np.asarray(c3, np.float32).reshape(INNER, DIM)
                [32 * c:32 * (c + 1)].reshape(1, V3_SL))
                * (W3_SCALE * A2_SCALE)).astype(BF16_NP)
        in_maps.append(m)
    return in_maps


def _run(inputs, trace=False, tmpdir=None):
    with_bias = (
        bool(np.any(inputs["b1"])) or bool(np.any(inputs["b2"])),
        bool(np.any(inputs["c1"])) or bool(np.any(inputs["c2"])),
        bool(np.any(inputs["b3"])),
        bool(np.any(inputs["c3"])),
    )
    nc = _get_program(with_bias)
    in_maps = _shard_inputs(with_bias=with_bias, **inputs)
    res = run_bass_kernel_spmd(nc, in_maps, core_ids=list(range(NCORES)),
                               trace=trace, tmpdir=tmpdir)
    outs = []
    for c in range(NCORES):
        yt = np.asarray(res.results[c]["yT"], dtype=np.float32)  # [2,128,600]
        outs.append(yt.transpose(0, 2, 1))  # [2, 600, 128]
    full = np.ascontiguousarray(np.concatenate(outs, axis=0)).astype(np.float32)
    return full, res


def kernel(**inputs) -> np.ndarray:
    out, _ = _run(inputs, trace=False)
    return out


# revision 37
# speedup vs baseline: 1.1409x; 1.0695x over previous
"""Trainium2 Bass kernel for nn_Attention_86423331930617.

Reference math (per batch b of 16):
  frate = [framerate[b], resolution[b]]                       # [2]
  h  = ((frate@W1+b1)@W2+b2)@W3+b3                            # [98304]
  qkvw = softmax(h.reshape(128, 768), axis=0)                 # over dim d
  wq, wk, wv = split(qkvw, 3, -1)                             # [128, 256] each
  q/k/v = x[b] @ w*      -> heads [8, 600, 32]
  dots = q@k.T / sqrt(32); attn = softmax(dots, -1) * mask[b]
  out  = attn @ v -> [600, 256]
  ow   = softmax(((frate@V1+c1)@V2+c2)@V3+c3 .reshape(256,128), axis=0)
  y    = out @ ow                                             # [600, 128]

Distribution over 8 cores (single NEFF, three AllToAlls):
  - warmup 256B AllToAll at t~0 absorbs the collective rendezvous
    barrier while the hypernet computes.
  - Hypernet big matmuls column-sharded (fp8 DoubleRow ks-pairs);
    A2A #1 ships q/k right after the 16 qk chunks; A2A #2 ships wv + ow
    and overlaps attention part 1.
  - Attention batch-sharded: core c does batches 2c, 2c+1.
DMA discipline: hypernet stores issue on the gpsimd queue (in front of
the collectives they feed), x/mask loads on the vector queue, weights +
a2a unpacks on sync. Unpacks/mask are single strided DMAs.
Packing tricks (tile_position inferred from out-AP base partition):
  - hypernet h-chunks: 4x [16,512] packed per PSUM bank -> 1 ACT copy
  - rowsums: ones[m,32] stationary -> [32,n] broadcast, 4 heads/bank
  - attn out (po): 4 heads/bank -> single TT applies 1/den and lands
    directly in the y-matmul operand layout
Softmax normalizers fold into ACT scales; exp needs no max-subtraction
(hypernet outputs and dots are O(0.1) by construction).
"""
import sys

sys.path.insert(0, "/opt/trn_rl_repo")
if "/root/.axon_site" not in sys.path:
    sys.path.insert(0, "/root/.axon_site")

import numpy as np
import ml_dtypes

import concourse.bass as bass
import concourse.mybir as mybir
import concourse.tile as tile
from concourse.vector_clock import ScopedClock
from concourse.bass_utils import run_bass_kernel_spmd

F32 = mybir.dt.float32
BF16 = mybir.dt.bfloat16
BF16_NP = ml_dtypes.bfloat16
FP8 = mybir.dt.float8e4
FP8_NP = mybir.dt.np(mybir.dt.float8e4)
DR = mybir.MatmulPerfMode.DoubleRow
W3_SCALE = 64.0
A2_SCALE = 4.0
UNSCALE = 1.0 / (W3_SCALE * A2_SCALE)
EXP = mybir.ActivationFunctionType.Exp
IDENT = mybir.ActivationFunctionType.Identity
MULT = mybir.AluOpType.mult

NCORES = 8
B, N, DIM, HEADS, DH = 16, 600, 128, 8, 32
INNER = HEADS * DH          # 256
D3 = 3 * DIM                # 384
E3 = 3 * INNER              # 768
BPC = B // NCORES           # batches per core = 2
W3_SL = 16 * E3             # 12288 w3 cols per core (16 d-rows)
V3_SL = 32 * DIM            # 4096 v3 cols per core (32 i-rows)
SCALE = DH ** -0.5
NH = [(0, 300), (300, 300)]                       # n halves
MT = [(0, 128), (128, 128), (256, 128), (384, 128), (512, 88)]  # m tiles
CH = 512                    # hypernet chunk cols
QK_W = 16 * 512             # 8192: q/k cols per core slice (a2a #1)
VB_W = 16 * 256             # 4096: wv cols per core slice (a2a #2)
A2_W = VB_W + V3_SL         # 8192: a2a #2 width (wv + ow)


# ---------------------------------------------------------------------------
# This walrus build accepts at most ONE sync wait / update per instruction;
# Tile can emit more. Split extras onto adjacent same-engine NoOps.
class _SplitWaitTileContext(tile.TileContext):
    def _split_sync(self, insts):
        out = []
        for inst in insts:
            si = inst.sync_info
            if si is None:
                out.append(inst)
                continue
            waits = list(si.on_wait) if si.on_wait else []
            updates = list(si.on_update) if si.on_update else []
            if len(waits) <= 1 and len(updates) <= 1:
                out.append(inst)
                continue
            for w in waits[1:]:
                nop = mybir.InstNoOp(name=f"I-{self.nc.next_id()}", ins=[], outs=[])
                nop.engine = inst.engine
                nop.sync_info = mybir.SyncInfo(on_wait=[w], on_update=[])
                out.append(nop)
            inst.sync_info = mybir.SyncInfo(on_wait=waits[:1], on_update=updates[:1])
            out.append(inst)
            for u in updates[1:]:
                nop = mybir.InstNoOp(name=f"I-{self.nc.next_id()}", ins=[], outs=[])
                nop.engine = inst.engine
                nop.sync_info = mybir.SyncInfo(on_wait=[], on_update=[u])
                out.append(nop)
        return out

    def _lower_ordered_insts(self, ordered):
        for bb_name in list(ordered.keys()):
            ordered[bb_name] = self._split_sync(ordered[bb_name])
        return super()._lower_ordered_insts(ordered)

    def _drain_and_barrier(self, tick_clock, wait_clock):
        nc = self.nc
        probe = nc.sync.nop()
        wait_clock.add_sem_waits(probe.ins, ScopedClock({None: tick_clock.global_clock}))
        si = probe.ins.sync_info
        waits = list(si.on_wait) if si is not None and si.on_wait else []
        if len(waits) > 1:
            probe.ins.sync_info = mybir.SyncInfo(on_wait=waits[:1], on_update=[])
            for w in waits[1:]:
                extra = nc.sync.nop()
                extra.ins.sync_info = mybir.SyncInfo(on_wait=[w], on_update=[])
        nc.sync.drain()
        nc.all_engine_barrier()
        assert self.sems is not None
        popped = nc._tile_sem_poison_stack.pop()
        assert popped is self._sem_poison
        nc.clear_and_free_semaphores(list(self.sems.allocated().values()))
        nc.all_engine_barrier()


# ---------------------------------------------------------------------------
def _build_program(with_bias):
    """Emit the per-core SPMD program. with_bias: (b12, c12, b3, c3) flags."""
    b12, c12, b3f, c3f = with_bias
    nc = bass.Bass("TRN2", target_bir_lowering=False, debug=False,
                   num_devices=NCORES)

    xT = nc.dram_tensor("xT", [BPC, DIM, N], BF16, kind="ExternalInput")
    maskT = nc.dram_tensor("maskT", [BPC, 5, DIM, N], BF16, kind="ExternalInput")
    frateT = nc.dram_tensor("frateT", [2, B], F32, kind="ExternalInput")
    w1 = nc.dram_tensor("w1", [2, D3], F32, kind="ExternalInput")
    w2 = nc.dram_tensor("w2", [D3, D3], F32, kind="ExternalInput")
    w3c = nc.dram_tensor("w3c", [D3, W3_SL], FP8, kind="ExternalInput")
    v1 = nc.dram_tensor("v1", [2, INNER], F32, kind="ExternalInput")
    v2 = nc.dram_tensor("v2", [INNER, INNER], F32, kind="ExternalInput")
    v3c = nc.dram_tensor("v3c", [INNER, V3_SL], FP8, kind="ExternalInput")
    if b12:
        b1t = nc.dram_tensor("b1t", [D3, 1], F32, kind="ExternalInput")
        b2t = nc.dram_tensor("b2t", [D3, 1], F32, kind="ExternalInput")
    if c12:
        c1t = nc.dram_tensor("c1t", [INNER, 1], F32, kind="ExternalInput")
        c2t = nc.dram_tensor("c2t", [INNER, 1], F32, kind="ExternalInput")
    if b3f:
        b3c = nc.dram_tensor("b3c", [1, W3_SL], BF16, kind="ExternalInput")
    if c3f:
        c3c = nc.dram_tensor("c3c", [1, V3_SL], BF16, kind="ExternalInput")
    yT = nc.dram_tensor("yT", [BPC, DIM, N], BF16, kind="ExternalOutput")

    with _SplitWaitTileContext(nc) as tc:
        with (
            tc.tile_pool(name="const", bufs=1) as cpool,
            tc.tile_pool(name="wts", bufs=1) as wpool,
            tc.tile_pool(name="achain", bufs=1) as apool,
            tc.tile_pool(name="hcopy", bufs=6) as hpool,
            tc.tile_pool(name="batch", bufs=1) as bpool,
            tc.tile_pool(name="etile", bufs=30) as epool,
            tc.tile_pool(name="rden", bufs=8) as rpool,
            # PSUM budget (8 banks of 2KB/partition):
            #   psA 3x2 banks (dots/a-chain/proj/y), psH 1 (hypernet chunk
            #   strips; ctx-freed after phase A for psD), psB 1 (rowsum
            #   packs, po packs, colsums; ping-pongs with psD in attention)
            tc.tile_pool(name="psA", bufs=3, space="PSUM") as psA,
            tc.tile_pool(name="psB", bufs=1, space="PSUM") as psB,
            tc.tile_pool(name="dram", bufs=1, space="DRAM") as dpool,
        ):
            # ---- constants
            ones_col = cpool.tile([DIM, 1], BF16, name="ones_col")
            nc.vector.memset(ones_col[:], 1.0)
            ones32 = cpool.tile([DIM, 32], BF16, name="ones32")
            nc.vector.memset(ones32[:], 1.0)
            if b3f or c3f:
                ones_row16 = cpool.tile([1, B], BF16, name="ones_row16")
                nc.vector.memset(ones_row16[:], 1.0)

            # ---- small weights in (sync queue)
            fr_sb = wpool.tile([2, B], F32, name="fr_sb")
            nc.sync.dma_start(out=fr_sb[:], in_=frateT[:])
            w1_sb = wpool.tile([2, D3], F32, name="w1_sb")
            nc.sync.dma_start(out=w1_sb[:], in_=w1[:])
            w2_sb = wpool.tile([DIM, 3, D3], F32, name="w2_sb")
            nc.sync.dma_start(out=w2_sb[:],
                              in_=w2[:].rearrange("(k p) n -> p k n", p=DIM))
            v1_sb = wpool.tile([2, INNER], F32, name="v1_sb")
            nc.sync.dma_start(out=v1_sb[:], in_=v1[:])
            v2_sb = wpool.tile([DIM, 2, INNER], F32, name="v2_sb")
            nc.sync.dma_start(out=v2_sb[:],
                              in_=v2[:].rearrange("(k p) n -> p k n", p=DIM))
            bias_sb = {}
            if b12:
                bias_sb["b1"] = wpool.tile([D3, 1], F32, name="b1_sb")
                nc.sync.dma_start(out=bias_sb["b1"][:], in_=b1t[:])
                bias_sb["b2"] = wpool.tile([D3, 1], F32, name="b2_sb")
                nc.sync.dma_start(out=bias_sb["b2"][:], in_=b2t[:])
            if c12:
                bias_sb["c1"] = wpool.tile([INNER, 1], F32, name="c1_sb")
                nc.sync.dma_start(out=bias_sb["c1"][:], in_=c1t[:])
                bias_sb["c2"] = wpool.tile([INNER, 1], F32, name="c2_sb")
                nc.sync.dma_start(out=bias_sb["c2"][:], in_=c2t[:])
            if b3f:
                b3_sb = wpool.tile([1, W3_SL], BF16, name="b3_sb")
                nc.sync.dma_start(out=b3_sb[:], in_=b3c[:])
            if c3f:
                c3_sb = wpool.tile([1, V3_SL], BF16, name="c3_sb")
                nc.sync.dma_start(out=c3_sb[:], in_=c3c[:])

            # ---- big hypernet weight slices (freed before phase B needs
            # the masked-exp tile pool)
            w3ctx = tc.tile_pool(name="w3", bufs=1)
            w3pool = w3ctx.__enter__()
            w3dr = w3pool.tile([DIM, 3, W3_SL], FP8, name="w3dr")
            w3view = w3c[:].rearrange("(ks p) n -> p ks n", p=DIM)
            for q4 in range(4):
                q0 = q4 * (W3_SL // 4)
                nc.sync.dma_start(out=w3dr[:, :, q0:q0 + W3_SL // 4],
                                  in_=w3view[:, :, q0:q0 + W3_SL // 4])
            v3dr = w3pool.tile([DIM, 2, V3_SL], FP8, name="v3dr")
            nc.sync.dma_start(
                out=v3dr[:], in_=v3c[:].rearrange("(ks p) n -> p ks n", p=DIM))

            # ---- a-chain: a1T = (frate@W1+b1).T as 3x[128,16]
            a1T = []
            for t in range(3):
                p = psA.tile([DIM, 1024], F32, name="pa", tag="pA")
                nc.tensor.matmul(p[:, :B], w1_sb[:, 128 * t:128 * (t + 1)],
                                 fr_sb[:], start=True, stop=True)
                s = apool.tile([DIM, B], F32, name=f"a1T{t}")
                if b12:
                    nc.scalar.activation(s[:], p[:, :B], IDENT,
                                         bias=bias_sb["b1"][128 * t:128 * (t + 1), :])
                else:
                    nc.scalar.copy(s[:], p[:, :B])
                a1T.append(s)
            a2f8 = apool.tile([DIM, 3, B], FP8, name="a2f8")
            for t in range(3):
                p = psA.tile([DIM, 1024], F32, name="pa2", tag="pA")
                for k in range(3):
                    nc.tensor.matmul(p[:, :B], w2_sb[:, k, 128 * t:128 * (t + 1)],
                                     a1T[k][:], start=(k == 0), stop=(k == 2))
                if b12:
                    tmp = apool.tile([DIM, B], F32, name=f"a2tmp{t}")
                    nc.scalar.activation(tmp[:], p[:, :B], IDENT,
                                         bias=bias_sb["b2"][128 * t:128 * (t + 1), :])
                    with nc.allow_low_precision("fp8 hypernet activations"):
                        nc.vector.tensor_scalar_mul(a2f8[:, t, :], tmp[:],
                                                    A2_SCALE)
                else:
                    nc.scalar.mul(a2f8[:, t, :], p[:, :B], A2_SCALE)
            av1T = []
            for t in range(2):
                p = psA.tile([DIM, 1024], F32, name="pav", tag="pA")
                nc.tensor.matmul(p[:, :B], v1_sb[:, 128 * t:128 * (t + 1)],
                                 fr_sb[:], start=True, stop=True)
                s = apool.tile([DIM, B], F32, name=f"av1T{t}")
                if c12:
                    nc.scalar.activation(s[:], p[:, :B], IDENT,
                                         bias=bias_sb["c1"][128 * t:128 * (t + 1), :])
                else:
                    nc.scalar.copy(s[:], p[:, :B])
                av1T.append(s)
            avf8 = apool.tile([DIM, 2, B], FP8, name="avf8")
            for t in range(2):
                p = psA.tile([DIM, 1024], F32, name="pav2", tag="pA")
                for k in range(2):
                    nc.tensor.matmul(p[:, :B], v2_sb[:, k, 128 * t:128 * (t + 1)],
                                     av1T[k][:], start=(k == 0), stop=(k == 1))
                if c12:
                    tmp = apool.tile([DIM, B], F32, name=f"avtmp{t}")
                    nc.scalar.activation(tmp[:], p[:, :B], IDENT,
                                         bias=bias_sb["c2"][128 * t:128 * (t + 1), :])
                    with nc.allow_low_precision("fp8 hypernet activations"):
                        nc.vector.tensor_scalar_mul(avf8[:, t, :], tmp[:],
                                                    A2_SCALE)
                else:
                    nc.scalar.mul(avf8[:, t, :], p[:, :B], A2_SCALE)

            # ---- x / mask inputs on the scalar queue, emitted after the
            # a-chain so its ACT ops aren't queued behind DMA dispatches
            xT_sb = [bpool.tile([DIM, N], BF16, name=f"xT_sb{i}")
                     for i in range(BPC)]
            for i in range(BPC):
                nc.scalar.dma_start(out=xT_sb[i][:], in_=xT[i])
            maskT_sb = [bpool.tile([DIM, 5, N], BF16, name=f"mask_sb{i}")
                        for i in range(BPC)]
            for i in range(BPC):
                nc.scalar.dma_start(
                    out=maskT_sb[i][:],
                    in_=maskT[i].rearrange("mt p n -> p mt n"))

            # ---- big hypernet matmuls -> two a2a inputs
            # host reorders w3c cols: first 16x512 (d-major, e<512 = q,k),
            # then 16x256 (e>=512 = wv). a2a #1 ships q/k right after the
            # qk chunks; a2a #2 ships wv + ow and overlaps attention part 1.
            # Chunks packed 4-per-bank at partition strips {0,32,64,96};
            # one ACT copy per 4 chunks. Output stays scaled by 1/UNSCALE;
            # consumers fold UNSCALE into their exp() scale. Stores issue on
            # the gpsimd queue in front of the collectives they feed.
            a2a1_in = dpool.tile([B, QK_W], BF16, name="a2a1_in")
            a2a2_in = dpool.tile([B, A2_W], BF16, name="a2a2_in")
            psHctx = tc.tile_pool(name="psH", bufs=1, space="PSUM")
            psH = psHctx.__enter__()

            def w3_group(g, tgt, toff, n_chunks=4):
                """Chunks 4g..4g+n_chunks of the 24 w3 chunks (512 cols)."""
                p = psH.tile([128, CH], F32, name="ph", tag="pH")
                for c in range(n_chunks):
                    j = 4 * g + c
                    st = 32 * c
                    for ks in range(3):
                        nc.tensor.matmul(p[st:st + B, :], a2f8[:, ks, :],
                                         w3dr[:, ks, CH * j:CH * (j + 1)],
                                         start=(ks == 0),
                                         stop=(ks == 2 and not b3f),
                                         tile_position=(0, st))
                    if b3f:
                        nc.tensor.matmul(p[st:st + B, :], ones_row16[:],
                                         b3_sb[:, CH * j:CH * (j + 1)],
                                         start=False, stop=True,
                                         tile_position=(0, st))
                s = hpool.tile([128, CH], BF16, name="hs", tag="hs")
                nc.scalar.copy(s[:], p[:])
                for c in range(n_chunks):
                    j = 4 * g + c
                    nc.gpsimd.dma_start(
                        out=tgt[:, CH * j - toff:CH * (j + 1) - toff],
                        in_=s[32 * c:32 * c + B, :])

            for g in range(4):          # qk chunks 0..15
                w3_group(g, a2a1_in, 0)
            a2a1_out = dpool.tile([B, QK_W], BF16, name="a2a1_out")
            nc.gpsimd.collective_compute(
                "AllToAll", mybir.AluOpType.bypass,
                replica_groups=[list(range(NCORES))],
                ins=[a2a1_in[:]], outs=[a2a1_out[:]],
            )
            for g in range(4, 6):       # wv chunks 16..23 -> a2a2 cols 0..4096
                w3_group(g, a2a2_in, QK_W)
            for g in range(2):          # v3: 8 chunks of 512 -> a2a #2 (ow)
                p = psH.tile([128, CH], F32, name="phv", tag="pH")
                for c in range(4):
                    j = 4 * g + c
                    st = 32 * c
                    for ks in range(2):
                        nc.tensor.matmul(p[st:st + B, :], avf8[:, ks, :],
                                         v3dr[:, ks, CH * j:CH * (j + 1)],
                                         start=(ks == 0),
                                         stop=(ks == 1 and not c3f),
                                         tile_position=(0, st))
                    if c3f:
                        nc.tensor.matmul(p[st:st + B, :], ones_row16[:],
                                         c3_sb[:, CH * j:CH * (j + 1)],
                                         start=False, stop=True,
                                         tile_position=(0, st))
                s = hpool.tile([128, CH], BF16, name="hvs", tag="hs")
                nc.scalar.copy(s[:], p[:])
                for c in range(4):
                    j = 4 * g + c
                    nc.gpsimd.dma_start(
                        out=a2a2_in[:, VB_W + CH * j:VB_W + CH * (j + 1)],
                        in_=s[32 * c:32 * c + B, :])
            psHctx.__exit__(None, None, None)
            # psH's bank is free once phase A drains; attention ping-pongs
            # rowsum/po packs between psB and psD so a pack's matmuls never
            # wait on the previous pack's DVE drain.
            psDctx = tc.tile_pool(name="psD", bufs=1, space="PSUM")
            psD = psDctx.__enter__()
            a2a2_out = dpool.tile([B, A2_W], BF16, name="a2a2_out")
            nc.gpsimd.collective_compute(
                "AllToAll", mybir.AluOpType.bypass,
                replica_groups=[list(range(NCORES))],
                ins=[a2a2_in[:]], outs=[a2a2_out[:]],
            )
            w3ctx.__exit__(None, None, None)
            emctx = tc.tile_pool(name="emk", bufs=62)
            em_pool = emctx.__enter__()
            # row (2s+i) holds my batch i's hypernet outputs from source s
            h1view = a2a1_out[:].rearrange(
                "(s two) (d e) -> two s d e", two=BPC, d=16)
            h2view = a2a2_out[:, :VB_W].rearrange(
                "(s two) (d e) -> two s d e", two=BPC, d=16)
            hvview = a2a2_out[:, VB_W:].rearrange(
                "(s two) (iv dd) -> two s iv dd", two=BPC, iv=32)

            # ================= attention =================
            # part1(b0) -> part2-pre(b0) -> part1(b1) with b0's attn@v
            # chains injected between heads (fills PE gaps while ACT paces
            # the exps) -> y(b0) -> part2(b1).
            st_ = [dict() for _ in range(BPC)]

            def part1(i, inject=None):
                s_i = st_[i]
                # lead-in pipelined by e-half: the q half (packs 0,1) flows
                # dma->exp->colsum->recip->proj before the k half's exp.
                qrawA = bpool.tile([DIM, 512], BF16, name="qrawA", tag="qrawA")
                for eh in range(2):
                    nc.gpsimd.dma_start(
                        out=qrawA[:, 256 * eh:256 * (eh + 1)],
                        in_=h1view[i][:, :, 256 * eh:256 * (eh + 1)])
                ehqA = bpool.tile([DIM, 512], BF16, name=f"ehqA{i}",
                                  tag=f"ehqA{i}")
                pcs = psB.tile([DIM, 512], F32, name="pcs", tag="pB")
                # recipA col p = softmax denom recip for e in [128p,128p+128)
                # == per-partition scale for qkT pack p (strips match).
                recipA = bpool.tile([DIM, 4], F32, name="recipA", tag="recipA")
                qkT = [None] * 4

                def lead_half(eh):
                    nc.scalar.activation(ehqA[:, 256 * eh:256 * (eh + 1)],
                                         qrawA[:, 256 * eh:256 * (eh + 1)],
                                         EXP, scale=UNSCALE)
                    for j in (2 * eh, 2 * eh + 1):
                        nc.tensor.matmul(pcs[:, j:j + 1],
                                         ehqA[:, 128 * j:128 * (j + 1)],
                                         ones_col[:], start=True, stop=True)
                    nc.vector.reciprocal(recipA[:, 2 * eh:2 * eh + 2],
                                         pcs[:, 2 * eh:2 * eh + 2])
                    if eh == 0:
                        nc.vector.tensor_scalar_mul(recipA[:, 0:2],
                                                    recipA[:, 0:2], SCALE)

                def proj_pack(pk):
                    pp = psA.tile([DIM, 1024], F32, name="pproj", tag="pA")
                    ppv = pp[:].rearrange("p (b n) -> p b n", b=2)
                    for st in range(2):
                        blk = 2 * pk + st
                        for hf, (n0, nsz) in enumerate(NH):
                            nc.tensor.matmul(
                                ppv[64 * st:64 * st + 64, hf, :nsz],
                                ehqA[:, 64 * blk:64 * blk + 64],
                                xT_sb[i][:, n0:n0 + nsz],
                                start=True, stop=True,
                                tile_position=(0, 64 * st))
                    s = bpool.tile([DIM, N], BF16, name=f"qkT{pk}",
                                   tag=f"qkT{i}_{pk}")
                    sview = s[:].rearrange("p (hf n) -> p hf n", hf=2)
                    nc.scalar.activation(
                        sview,
                        pp[:].rearrange("p (hf n) -> p hf n", hf=2)[:, :, :300],
                        mybir.ActivationFunctionType.Copy,
                        scale=recipA[:, pk:pk + 1])
                    qkT[pk] = s

                # heads 0-3 consume packs (0, 2): emit those first
                lead_half(0)
                proj_pack(0)
                lead_half(1)
                proj_pack(2)
                proj_pack(1)
                proj_pack(3)
                em_all, em_eun, rden_all = {}, {}, {}
                s_i["em"], s_i["rden"] = em_all, rden_all

                def emit_rowsum_burst(kg):
                    # 20 contiguous full-K PE matmuls per half
                    for hf, (n0, nsz) in enumerate(NH):
                        pool_, tag_ = ((psB, "pB") if (2 * kg + hf) % 2 == 0
                                       else (psD, "pD"))
                        prs = pool_.tile([128, 512], F32, name="prs", tag=tag_)
                        for j in range(4):
                            h = 4 * kg + j
                            for mt, (m0, msz) in enumerate(MT):
                                nc.tensor.matmul(
                                    prs[32 * j:32 * j + 32, :nsz],
                                    ones32[:msz, :],
                                    em_eun[(h, mt)][:msz, n0:n0 + nsz],
                                    start=(mt == 0), stop=(mt == 4),
                                    tile_position=(0, 32 * j))
                        rden = rpool.tile([128, 300], F32, name="rden",
                                          tag="rden")
                        nc.vector.reciprocal(rden[:], prs[:, :300])
                        rden_all[(kg, hf)] = rden

                for h in range(HEADS):
                    pk, ro = h // 4, 32 * (h % 4)
                    qp, kp = qkT[pk], qkT[2 + pk]
                    for mt, (m0, msz) in enumerate(MT):
                        pd = psA.tile([128, 1024], F32, name="pdots", tag="pA")
                        pdv = pd[:].rearrange("p (b n) -> p b n", b=2)
                        for hf, (n0, nsz) in enumerate(NH):
                            nc.tensor.matmul(
                                pdv[:msz, hf, :nsz],
                                kp[ro:ro + 32, m0:m0 + msz],
                                qp[ro:ro + 32, n0:n0 + nsz],
                                start=True, stop=True,
                                tile_position=(ro, 0))
                        e_t = epool.tile([128, N], BF16, name="e_t", tag="e")
                        nc.scalar.activation(
                            e_t[:msz].rearrange("p (hf n) -> p hf n", hf=2),
                            pd[:msz].rearrange(
                                "p (hf n) -> p hf n", hf=2)[:, :, :300],
                            EXP)
                        em_t = em_pool.tile([128, N], BF16, name="em_t",
                                            tag="em")
                        nc.vector.tensor_mul(em_t[:msz], e_t[:msz],
                                             maskT_sb[i][:msz, mt, :])
                        em_eun[(h, mt)] = e_t
                        em_all[(h, mt)] = em_t
                    if h == 4:
                        emit_rowsum_burst(0)
                    if inject is not None:
                        inject(h)
                emit_rowsum_burst(1)

            def part2_fetch(i):
                # a2a2 unpack + exps + colsums + normalizer fold: injected
                # into part1's head loop so this chain never exposes its
                # latency at the part1 -> part2 transition.
                s_i = st_[i]
                qrawB = bpool.tile([DIM, 256], BF16, name="qrawB", tag="qrawB")
                nc.gpsimd.dma_start(out=qrawB[:], in_=h2view[i])
                ehqB = bpool.tile([DIM, 256], BF16, name=f"ehqB{i}",
                                  tag=f"ehqB{i}")
                nc.scalar.activation(ehqB[:], qrawB[:], EXP, scale=UNSCALE)
                # ow: assemble exp(v3h) as 2x[128e, 128d]
                ehvB = []
                for k in range(2):
                    vr = bpool.tile([DIM, DIM], BF16, name=f"vraw{k}",
                                    tag=f"vraw{k}")
                    nc.gpsimd.dma_start(out=vr[:],
                                        in_=hvview[i, 4 * k:4 * k + 4])
                    ev = bpool.tile([DIM, DIM], BF16, name=f"ehvB{k}",
                                    tag=f"ehvB{i}_{k}")
                    nc.scalar.activation(ev[:], vr[:], EXP, scale=UNSCALE)
                    ehvB.append(ev)
                # colsums: wv normalizer (per e) cols 0,1; ow S_d col 2
                pcs2 = psB.tile([DIM, 512], F32, name="pcs2", tag="pB")
                for j in range(2):
                    nc.tensor.matmul(pcs2[:, j:j + 1],
                                     ehqB[:, 128 * j:128 * (j + 1)],
                                     ones_col[:], start=True, stop=True)
                for k in range(2):
                    nc.tensor.matmul(pcs2[:, 2:3], ehvB[k][:], ones_col[:],
                                     start=(k == 0), stop=(k == 1))
                recipB = bpool.tile([DIM, 4], F32, name=f"recipB{i}",
                                    tag=f"recipB{i}")
                nc.vector.reciprocal(recipB[:, 0:3], pcs2[:, 0:3])
                # fold wv normalizer (per e-row) into ow rows
                for k in range(2):
                    nc.vector.tensor_scalar_mul(ehvB[k][:], ehvB[k][:],
                                                recipB[:, k:k + 1])
                s_i["ehvB"], s_i["recipB"], s_i["ehqB"] = ehvB, recipB, ehqB

            def part2_pre(i):
                s_i = st_[i]
                ehqB = s_i["ehqB"]
                # v = x @ exp(wv) (unnormalized; fixed via ehvB rows in fetch)
                v_sb = []
                for mt, (m0, msz) in enumerate(MT):
                    pv = psA.tile([DIM, 1024], F32, name="pv", tag="pA")
                    nc.tensor.matmul(pv[:msz, :INNER],
                                     xT_sb[i][:, m0:m0 + msz],
                                     ehqB[:], start=True, stop=True)
                    s = bpool.tile([128, INNER], BF16, name=f"v_sb{mt}",
                                   tag=f"v_sb{i}_{mt}")
                    nc.scalar.copy(s[:msz, :], pv[:msz, :INNER])
                    v_sb.append(s)
                s_i["v"] = v_sb
                s_i["outTB"] = [bpool.tile([DIM, N], BF16, name=f"outTB{k}",
                                           tag=f"outTB{i}_{k}")
                                for k in range(2)]

            def part2_po(i, kg, hf):
                # attn@v pack: 4 heads/bank; TT applies 1/den and lands in
                # outTB[kg] (e-rows) for the y matmul.
                s_i = st_[i]
                n0, nsz = NH[hf]
                pool_, tag_ = ((psB, "pB") if (2 * kg + hf) % 2 == 0
                               else (psD, "pD"))
                po = pool_.tile([128, 512], F32, name="po", tag=tag_)
                for j in range(4):
                    h = 4 * kg + j
                    for mt, (m0, msz) in enumerate(MT):
                        nc.tensor.matmul(
                            po[32 * j:32 * j + 32, :nsz],
                            s_i["v"][mt][:msz, 32 * h:32 * h + 32],
                            s_i["em"][(h, mt)][:msz, n0:n0 + nsz],
                            start=(mt == 0), stop=(mt == 4),
                            tile_position=(0, 32 * j))
                with nc.allow_low_precision("attn out bf16"):
                    nc.vector.tensor_mul(s_i["outTB"][kg][:, n0:n0 + nsz],
                                         po[:, :nsz],
                                         s_i["rden"][(kg, hf)][:])

            def part2_y(i):
                # split by n-half so the copy/store of half 0 overlaps the
                # half-1 matmuls (shaves the kernel tail)
                s_i = st_[i]
                py = psA.tile([DIM, 1024], F32, name="py", tag="pA")
                pyv = py[:].rearrange("p (b n) -> p b n", b=2)
                ys = bpool.tile([DIM, N], BF16, name="ys", tag=f"ys{i}")
                for hf, (n0, nsz) in enumerate(NH):
                    for k in range(2):
                        nc.tensor.matmul(
                            pyv[:, hf, :nsz], s_i["ehvB"][k][:],
                            s_i["outTB"][k][:, n0:n0 + nsz],
                            start=(k == 0), stop=(k == 1))
                    nc.scalar.activation(
                        ys[:, n0:n0 + nsz], pyv[:, hf, :nsz],
                        mybir.ActivationFunctionType.Copy,
                        scale=s_i["recipB"][:, 2:3])
                    nc.sync.dma_start(out=yT[i, :, n0:n0 + nsz],
                                      in_=ys[:, n0:n0 + nsz])

            def inject_f0(h):
                # b0's a2a2 fetch chain overlaps b0's part-1 tail
                if h == 6:
                    part2_fetch(0)

            part1(0, inject=inject_f0)
            part2_pre(0)

            def inject_b0(h):
                # b0's attn@v chains run in b1's part-1 PE gaps
                if h in (1, 3, 5, 7):
                    kg, hf = divmod((h - 1) // 2, 2)
                    part2_po(0, kg, hf)
                if h == 6:
                    part2_fetch(1)

            part1(1, inject=inject_b0)
            part2_y(0)
            part2_pre(1)
            for kg in range(2):
                for hf in range(2):
                    part2_po(1, kg, hf)
            part2_y(1)
            emctx.__exit__(None, None, None)
            psDctx.__exit__(None, None, None)

    return nc


_PROGRAM_CACHE = {}


def _get_program(with_bias):
    if with_bias not in _PROGRAM_CACHE:
        _PROGRAM_CACHE[with_bias] = _build_program(with_bias)
    return _PROGRAM_CACHE[with_bias]


def _shard_inputs(x, mask, resolution, framerate,
                  W1, b1, W2, b2, W3, b3, V1, c1, V2, c2, V3, c3, with_bias):
    b12, c12, b3f, c3f = with_bias
    x = np.asarray(x, np.float32)
    mask = np.asarray(mask, np.float32)
    xT = np.ascontiguousarray(x.transpose(0, 2, 1)).astype(BF16_NP)
    maskTn = np.ascontiguousarray(
        mask[0, :, 0].transpose(0, 2, 1)).astype(BF16_NP)     # [B, 600m, 600n]
    maskTp = np.zeros((B, 5 * DIM, N), BF16_NP)
    maskTp[:, :N, :] = maskTn
    maskTp = maskTp.reshape(B, 5, DIM, N)
    frateT = np.ascontiguousarray(
        np.stack([np.asarray(framerate, np.float32),
                  np.asarray(resolution, np.float32)], axis=0))
    W1 = np.ascontiguousarray(np.asarray(W1, np.float32))
    W2 = np.ascontiguousarray(np.asarray(W2, np.float32))
    V1 = np.ascontiguousarray(np.asarray(V1, np.float32))
    V2 = np.ascontiguousarray(np.asarray(V2, np.float32))
    W3v = np.asarray(W3, np.float32).reshape(D3, DIM, E3)
    V3v = np.asarray(V3, np.float32).reshape(INNER, INNER, DIM)
    in_maps = []
    for c in range(NCORES):
        m = {
            "xT": xT[BPC * c:BPC * (c + 1)],
            "maskT": maskTp[BPC * c:BPC * (c + 1)],
            "frateT": frateT,
            "w1": W1, "w2": W2, "v1": V1, "v2": V2,
            # reordered: (d-major, e<512) then (d-major, e>=512) — matches
            # the split-a2a chunk layout in the device program
            "w3c": (np.concatenate([
                W3v[:, 16 * c:16 * (c + 1), :512].reshape(D3, 16 * 512),
                W3v[:, 16 * c:16 * (c + 1), 512:].reshape(D3, 16 * 256),
            ], axis=1) * W3_SCALE).astype(FP8_NP),
            "v3c": (np.ascontiguousarray(
                V3v[:, 32 * c:32 * (c + 1), :]).reshape(INNER, V3_SL)
                * W3_SCALE).astype(FP8_NP),
        }
        if b12:
            m["b1t"] = np.asarray(b1, np.float32).reshape(D3, 1)
            m["b2t"] = np.asarray(b2, np.float32).reshape(D3, 1)
        if c12:
            m["c1t"] = np.asarray(c1, np.float32).reshape(INNER, 1)
            m["c2t"] = np.asarray(c2, np.float32).reshape(INNER, 1)
        if b3f:
            b3v = np.asarray(b3, np.float32).reshape(DIM, E3)[16 * c:16 * (c + 1)]
            m["b3c"] = (np.concatenate(
                [b3v[:, :512].reshape(1, 16 * 512),
                 b3v[:, 512:].reshape(1, 16 * 256)], axis=1)
                * (W3_SCALE * A2_SCALE)).astype(BF16_NP)
        if c3f:
            m["c3c"] = (np.ascontiguousarray(
                np.asarray(c3, np.float32).reshape(INNER, DIM)
                [32 * c:32 * (c + 1)].reshape(1, V3_SL))
                * (W3_SCALE * A2_SCALE)).astype(BF16_NP)
        in_maps.append(m)
    return in_maps


def _run(inputs, trace=False, tmpdir=None):
    with_bias = (
        bool(np.any(inputs["b1"])) or bool(np.any(inputs["b2"])),
        bool(np.any(inputs["c1"])) or bool(np.any(inputs["c2"])),
        bool(np.any(inputs["b3"])),
        bool(np.any(inputs["c3"])),
    )
    nc = _get_program(with_bias)
    in_maps = _shard_inputs(with_bias=with_bias, **inputs)
    res = run_bass_kernel_spmd(nc, in_maps, core_ids=list(range(NCORES)),
                               trace=trace, tmpdir=tmpdir)
    outs = []
    for c in range(NCORES):
        yt = np.asarray(res.results[c]["yT"], dtype=np.float32)  # [2,128,600]
        outs.append(yt.transpose(0, 2, 1))  # [2, 600, 128]
    full = np.ascontiguousarray(np.concatenate(outs, axis=0)).astype(np.float32)
    return full, res


def kernel(**inputs) -> np.ndarray:
    out, _ = _run(inputs, trace=False)
    return out
